# revision 1
# baseline (speedup 1.0000x reference)
"""GAT 2-layer GNN kernel for 8 Trainium2 NeuronCores.

Strategy (graph/data parallel, per the sharding hint):
  - Nodes (and their incident edges, keyed by dst) are partitioned into 8
    contiguous shards of 6250 nodes.
  - Each core computes its shard's node features h = x @ W1 plus the
    attention scalars a_src/a_dst, packs them into 256-byte table rows
    [h(bf16 x64) | a_src(f32 x8) | a_dst(f32 x8) | pad], and an AllGather
    replicates the full 50000-row table to every core (the "halo exchange" -
    with a uniformly random graph every boundary is shared).
  - Edges are laid out in ELL style: each core's dst nodes are grouped into
    blocks of 128 (degree-balanced via a lexicographic (lo-deg, hi-deg)
    sort), and an edge sits on SBUF partition = its dst's position in the
    block, chunk = its index within the dst's edge list. h[src]/a_src[src]
    rows are fetched per-edge with dma_gather (int16 indices, so the table
    is addressed from two bases: rows < 32768 and rows >= 32768); the
    block's own member rows (for a_dst[dst]) ride along as two extra
    chunks per block.
  - With dst == partition, the segment softmax-aggregate needs no one-hot
    matrix: R[e, :] = [exp(leakyrelu(a_src[src]+a_dst[dst])) * h_src | exp]
    (zeroed on padding slots via a host mask stream) and PSUM accumulates
    identity^T @ R over the block's chunks, i.e. a plain per-partition sum
    giving [numerator | denominator] per dst.
  - Layer 2 repeats the same pipeline (1 head, 40 channels) on the
    relu(out1) features (whose table rows are written in block order; the
    host translates gather indices accordingly), followed by a fused
    log_softmax. The host undoes the block permutation on the output.
"""

import os
import sys

sys.path.insert(0, "/opt/trn_rl_repo")

import numpy as np
import ml_dtypes

import concourse.bacc as bacc
import concourse.mybir as mybir
from concourse import tile
from concourse.bass_utils import run_bass_kernel_spmd
from concourse.masks import make_identity

bf16 = ml_dtypes.bfloat16

N_NODES = 50000
F_IN = 512
H1 = 8
HID = 8
D1 = H1 * HID  # 64
C2 = 40
N_CORES = 8
SHARD = N_NODES // N_CORES  # 6250
BLK = 128
NB = (SHARD + BLK - 1) // BLK  # 49 blocks per core (last has 106 dsts)
SPLIT = 32768  # int16 index range split for the gather table
SBG = 3  # blocks per gather super-group
NEG_SLOPE = 0.2
TROW = 128  # table row: 128 bf16 = 256 bytes

f32 = mybir.dt.float32
bfl = mybir.dt.bfloat16
i16 = mybir.dt.int16

_CACHE = {}


def _install_ntff_hook():
    """Provide antenv.axon_hooks if the image lacks it, driving NTFF
    profiling via the injected libaxon_pjrt.so C ABI (see trn_boot)."""
    try:
        from antenv.axon_hooks import get_axon_ntff_profile_hook  # noqa: F401
        return
    except ImportError:
        pass
    import contextlib
    import ctypes
    import types

    so_path = "/opt/axon/libaxon_pjrt.so"
    try:
        lib = ctypes.CDLL(so_path)
    except OSError:
        return
    if not hasattr(lib, "axon_start_nrt_profile"):
        return
    lib.axon_start_nrt_profile.argtypes = [ctypes.POINTER(ctypes.c_int64),
                                           ctypes.c_size_t]
    lib.axon_start_nrt_profile.restype = ctypes.c_int64
    lib.axon_stop_nrt_profile.argtypes = [ctypes.c_char_p]
    lib.axon_stop_nrt_profile.restype = ctypes.c_int64

    @contextlib.contextmanager
    def _hook(output_dir, device_ids):
        import jax
        jax.devices()
        if device_ids:
            ids = (ctypes.c_int64 * len(device_ids))(*device_ids)
            rc = lib.axon_start_nrt_profile(ids, len(device_ids))
        else:
            rc = lib.axon_start_nrt_profile(None, 0)
        if rc != 0:
            raise RuntimeError(f"axon_start_nrt_profile rc={rc}")
        try:
            yield
        finally:
            n = lib.axon_stop_nrt_profile(str(output_dir).encode())
            print(f"ntff profile: {n} file(s) written to {output_dir}")

    import antenv
    mod = types.ModuleType("antenv.axon_hooks")
    mod.get_axon_ntff_profile_hook = lambda: _hook
    mod.set_axon_ntff_profile_hook = lambda h: None
    sys.modules["antenv.axon_hooks"] = mod
    antenv.axon_hooks = mod


def _ceil(a, b):
    return (a + b - 1) // b


class LayerPlan:
    """Host-side ELL layout for one layer's edge phase (all cores)."""


def _plan_layer(src_row, dst_node, row_of=None):
    """Build the ELL plan. src_row: per-edge gather row id in the table
    (layer specific); dst_node: per-edge global dst node id; row_of[c, n]:
    global table row of node n of core c (None -> node-ordered)."""
    plan = LayerPlan()
    core = dst_node // SHARD
    local = dst_node - core * SHARD
    hi = (src_row >= SPLIT).astype(np.int64)

    # per-(core,node) lo/hi degree
    klo = np.zeros((N_CORES, SHARD), np.int64)
    khi = np.zeros((N_CORES, SHARD), np.int64)
    np.add.at(klo, (core, local), 1 - hi)
    np.add.at(khi, (core, local), hi)

    # block membership per core: lexicographic (klo desc, khi desc) sort
    # -> blocks of 128 with similar lo/hi degrees
    perm = np.full((N_CORES, NB * BLK), -1, np.int64)
    order = np.lexsort((-khi, -klo), axis=-1)
    for c in range(N_CORES):
        perm[c, :SHARD] = order[c]
    slot_of = np.zeros((N_CORES, SHARD), np.int64)
    for c in range(N_CORES):
        slot_of[c, order[c]] = np.arange(SHARD)

    # per-block chunk counts, uniform across cores
    klo_pad = np.zeros((N_CORES, NB * BLK), np.int64)
    khi_pad = np.zeros((N_CORES, NB * BLK), np.int64)
    for c in range(N_CORES):
        klo_pad[c, :SHARD] = klo[c, order[c]]
        khi_pad[c, :SHARD] = khi[c, order[c]]
    nch_lo = np.maximum(klo_pad.reshape(N_CORES, NB, BLK).max(axis=(0, 2)), 1)
    nch_hi = np.maximum(khi_pad.reshape(N_CORES, NB, BLK).max(axis=(0, 2)), 1)

    # chunk layout per super-group:
    #   [lo runs of blocks][member-lo chunk per block][hi runs][member-hi]
    ngroups = _ceil(NB, SBG)
    lo_runs = [None] * NB
    hi_runs = [None] * NB
    mlo_ch = [0] * NB
    mhi_ch = [0] * NB
    groups = []
    goff = 0
    for g in range(ngroups):
        blocks = list(range(g * SBG, min((g + 1) * SBG, NB)))
        ch = goff
        for b in blocks:
            lo_runs[b] = (ch, int(nch_lo[b]))
            ch += int(nch_lo[b])
        for b in blocks:
            mlo_ch[b] = ch
            ch += 1
        nlo_ch = ch - goff
        for b in blocks:
            hi_runs[b] = (ch, int(nch_hi[b]))
            ch += int(nch_hi[b])
        for b in blocks:
            mhi_ch[b] = ch
            ch += 1
        groups.append((blocks, goff, ch - goff, nlo_ch))
        goff = ch
    total_ch = goff

    # per-core streams
    lo_starts = np.array([r[0] for r in lo_runs], np.int64)
    hi_starts = np.array([r[0] for r in hi_runs], np.int64)
    idx_streams, mask_streams = [], []
    for c in range(N_CORES):
        sel = core == c
        e_row = src_row[sel]
        e_loc = local[sel]
        e_hi = hi[sel]
        e_slot = slot_of[c, e_loc]
        e_blk = e_slot // BLK
        e_p = e_slot % BLK
        key = e_slot * 2 + e_hi
        o = np.argsort(key, kind="stable")
        inv = np.empty_like(o)
        inv[o] = np.arange(len(o))
        e_pos = _running_count(key[o])[inv]
        e_ch = np.where(e_hi == 0, lo_starts[e_blk], hi_starts[e_blk]) + e_pos
        slots = e_ch * BLK + e_p

        idx = np.zeros(total_ch * BLK, np.int16)
        mask = np.zeros(total_ch * BLK, np.float32)
        idx[slots] = (e_row - e_hi * SPLIT).astype(np.int16)
        mask[slots] = 1.0
        for b in range(NB):
            mem = perm[c, b * BLK:(b + 1) * BLK]
            valid = mem >= 0
            if row_of is None:
                mrow = np.where(valid, mem + c * SHARD, 0)
            else:
                mrow = np.where(valid, row_of[c, mem.clip(0)], 0)
            is_lo = mrow < SPLIT
            s0 = mlo_ch[b] * BLK
            idx[s0:s0 + BLK] = np.where(valid & is_lo, mrow, 0).astype(np.int16)
            mask[s0:s0 + BLK] = (valid & is_lo).astype(np.float32)
            s1 = mhi_ch[b] * BLK
            idx[s1:s1 + BLK] = np.where(valid & ~is_lo, mrow - SPLIT,
                                        0).astype(np.int16)
            mask[s1:s1 + BLK] = (valid & ~is_lo).astype(np.float32)

        idx_w = np.tile(idx.reshape(total_ch * 8, 16).T, (8, 1)).copy()
        mask_w = mask.reshape(total_ch, BLK).T.astype(bf16).copy()
        idx_streams.append(idx_w)
        mask_streams.append(mask_w)

    plan.nch_lo = nch_lo
    plan.nch_hi = nch_hi
    plan.groups = groups
    plan.total_ch = total_ch
    plan.mlo_ch = mlo_ch
    plan.mhi_ch = mhi_ch
    plan.lo_runs = lo_runs
    plan.hi_runs = hi_runs
    plan.idx_streams = idx_streams
    plan.mask_streams = mask_streams
    plan.perm = perm
    return plan


def _running_count(k):
    """pos[i] = number of j<i with k[j]==k[i]; k is sorted."""
    n = len(k)
    if n == 0:
        return np.zeros(0, np.int64)
    starts = np.r_[0, np.flatnonzero(np.diff(k)) + 1]
    run_id = np.zeros(n, np.int64)
    run_id[starts[1:]] = 1
    run_id = np.cumsum(run_id)
    return np.arange(n) - starts[run_id]


def _prep(edge_index):
    src = np.asarray(edge_index[0], dtype=np.int64)
    dst = np.asarray(edge_index[1], dtype=np.int64)
    loops = np.arange(N_NODES, dtype=np.int64)
    src = np.concatenate([src, loops])
    dst = np.concatenate([dst, loops])

    # layer 1: table rows are node-ordered
    plan1 = _plan_layer(src, dst)

    # layer 2: table rows are block-slot-ordered per core
    s_core = src // SHARD
    s_local = src - s_core * SHARD
    slot_of1 = np.zeros((N_CORES, SHARD), np.int64)
    for c in range(N_CORES):
        slot_of1[c, plan1.perm[c, :SHARD]] = np.arange(SHARD)
    src_row2 = s_core * SHARD + slot_of1[s_core, s_local]
    row_of2 = slot_of1 + (np.arange(N_CORES) * SHARD)[:, None]
    plan2 = _plan_layer(src_row2, dst, row_of=row_of2)
    return plan1, plan2


def _build(plan1, plan2):
    nc = bacc.Bacc("TRN2", target_bir_lowering=False, debug=False,
                   num_devices=N_CORES, num_swdge_queues=2)

    NPADROWS = NB * BLK  # 6272 (last 22 cols scratch)
    xT_ext = nc.declare_dram_parameter("xT", [F_IN, NPADROWS], bfl, isOutput=False)
    w1_ext = nc.declare_dram_parameter("w1r", [128, 4 * D1], bfl, isOutput=False)
    w2_ext = nc.declare_dram_parameter("w2", [D1, C2], bfl, isOutput=False)
    a1s_ext = nc.declare_dram_parameter("a1srep", [128, D1], f32, isOutput=False)
    a1d_ext = nc.declare_dram_parameter("a1drep", [128, D1], f32, isOutput=False)
    a2s_ext = nc.declare_dram_parameter("a2srep", [128, C2], f32, isOutput=False)
    a2d_ext = nc.declare_dram_parameter("a2drep", [128, C2], f32, isOutput=False)
    b1_ext = nc.declare_dram_parameter("b1rep", [128, D1], f32, isOutput=False)
    b2_ext = nc.declare_dram_parameter("b2rep", [128, C2], f32, isOutput=False)
    idx1_ext = nc.declare_dram_parameter("idx1", [128, plan1.total_ch * 8], i16,
                                         isOutput=False)
    msk1_ext = nc.declare_dram_parameter("msk1", [128, plan1.total_ch], bfl,
                                         isOutput=False)
    idx2_ext = nc.declare_dram_parameter("idx2", [128, plan2.total_ch * 8], i16,
                                         isOutput=False)
    msk2_ext = nc.declare_dram_parameter("msk2", [128, plan2.total_ch], bfl,
                                         isOutput=False)
    out_ext = nc.declare_dram_parameter("out", [NB * BLK, C2], f32, isOutput=True)
    debug = os.environ.get("K_DEBUG", "0") == "1"
    dbg1_ext = (nc.declare_dram_parameter("dbg1", [NB * BLK, D1 + H1], f32,
                                          isOutput=True) if debug else None)

    t1_shard = nc.dram_tensor("t1_shard", [SHARD, TROW], bfl)
    t1_full = nc.dram_tensor("t1_full", [N_NODES, TROW], bfl, addr_space="Shared")
    t2_shard = nc.dram_tensor("t2_shard", [SHARD, TROW], bfl)
    t2_full = nc.dram_tensor("t2_full", [N_NODES, TROW], bfl, addr_space="Shared")

    rg = [list(range(N_CORES))]

    with tile.TileContext(nc) as tc:
        with tc.tile_pool(name="const", bufs=1) as cpool:
            ident = cpool.tile([128, 128], bfl)
            make_identity(nc, ident[:, :])
            a1s_t = cpool.tile([128, D1], f32)
            nc.sync.dma_start(out=a1s_t[:, :], in_=a1s_ext[:, :])
            a1d_t = cpool.tile([128, D1], f32)
            nc.sync.dma_start(out=a1d_t[:, :], in_=a1d_ext[:, :])
            a2s_t = cpool.tile([128, C2], f32)
            nc.sync.dma_start(out=a2s_t[:, :], in_=a2s_ext[:, :])
            a2d_t = cpool.tile([128, C2], f32)
            nc.sync.dma_start(out=a2d_t[:, :], in_=a2d_ext[:, :])
            b1_t = cpool.tile([128, D1], f32)
            nc.sync.dma_start(out=b1_t[:, :], in_=b1_ext[:, :])
            b2_t = cpool.tile([128, C2], f32)
            nc.sync.dma_start(out=b2_t[:, :], in_=b2_ext[:, :])
            w2_t = cpool.tile([D1, C2], bfl)
            nc.sync.dma_start(out=w2_t[:, :], in_=w2_ext[:, :])
            tab1_sb = cpool.tile([128, NB, TROW], bfl)
            tab2_sb = cpool.tile([128, NB, TROW], bfl)
            nc.vector.memset(tab1_sb[:, :, :], 0.0)
            nc.vector.memset(tab2_sb[:, :, :], 0.0)

            stage = int(os.environ.get("K_STAGE", "3"))

            # ---------------- Phase A/B: h1 = x @ W1, attention scalars ---
            with tc.tile_pool(name="phA", bufs=2) as apool, \
                 tc.tile_pool(name="phA_ps", bufs=2, space="PSUM") as apsum:
                w1_t = apool.tile([128, 4, D1], bfl, tag="w1")
                nc.sync.dma_start(out=w1_t[:, :, :], in_=w1_ext[:, :])
                xk = []
                for k in range(4):
                    xt = apool.tile([128, NPADROWS], bfl, tag=f"xk{k}")
                    nc.sync.dma_start(out=xt[:, :],
                                      in_=xT_ext[k * 128:(k + 1) * 128, :])
                    xk.append(xt)
                for b in range(NB):
                    hps = apsum.tile([128, D1], f32, tag="hps")
                    for k in range(4):
                        nc.tensor.matmul(
                            hps[:, :], lhsT=xk[k][:, b * BLK:(b + 1) * BLK],
                            rhs=w1_t[:, k, :], start=(k == 0), stop=(k == 3))
                    nc.scalar.activation(out=tab1_sb[:, b, 0:D1], in_=hps[:, :],
                                         func=mybir.ActivationFunctionType.Copy)
                    tmp = apool.tile([128, D1], f32, tag="atmp")
                    nc.vector.tensor_tensor(out=tmp[:, :], in0=hps[:, :],
                                            in1=a1s_t[:, :],
                                            op=mybir.AluOpType.mult)
                    nc.vector.tensor_reduce(
                        out=tab1_sb[:, b, 64:80].bitcast(f32),
                        in_=tmp[:, :].rearrange("p (h c) -> p h c", h=H1, c=HID),
                        axis=mybir.AxisListType.X, op=mybir.AluOpType.add)
                    tmp2 = apool.tile([128, D1], f32, tag="atmp2")
                    nc.vector.tensor_tensor(out=tmp2[:, :], in0=hps[:, :],
                                            in1=a1d_t[:, :],
                                            op=mybir.AluOpType.mult)
                    nc.vector.tensor_reduce(
                        out=tab1_sb[:, b, 80:96].bitcast(f32),
                        in_=tmp2[:, :].rearrange("p (h c) -> p h c", h=H1, c=HID),
                        axis=mybir.AxisListType.X, op=mybir.AluOpType.add)

            _dma_table_out(nc, t1_shard, tab1_sb)
            nc.gpsimd.collective_compute(
                "AllGather", mybir.AluOpType.bypass, replica_groups=rg,
                ins=[t1_shard.ap().opt()], outs=[t1_full.ap().opt()])

            if stage >= 2:
                _edge_phase(nc, tc, layer=1, table_full=t1_full,
                            idx_ext=idx1_ext, msk_ext=msk1_ext, ident=ident,
                            plan=plan1, a_s=a2s_t, a_d=a2d_t, bias=b1_t,
                            w2_t=w2_t, tab_out=tab2_sb, out_ext=None,
                            b2_t=None, dbg_ext=dbg1_ext)

                _dma_table_out(nc, t2_shard, tab2_sb)
                nc.gpsimd.collective_compute(
                    "AllGather", mybir.AluOpType.bypass, replica_groups=rg,
                    ins=[t2_shard.ap().opt()], outs=[t2_full.ap().opt()])

            if stage >= 3:
                _edge_phase(nc, tc, layer=2, table_full=t2_full,
                            idx_ext=idx2_ext, msk_ext=msk2_ext, ident=ident,
                            plan=plan2, a_s=None, a_d=None, bias=None,
                            w2_t=None, tab_out=None, out_ext=out_ext,
                            b2_t=b2_t, dbg_ext=None)
            else:
                with tc.tile_pool(name="stub", bufs=1) as spool:
                    z = spool.tile([128, C2], f32)
                    nc.vector.memset(z[:, :], 0.0)
                    for b in range(NB):
                        nc.sync.dma_start(out=out_ext[b * BLK:(b + 1) * BLK, :],
                                          in_=z[:, :])

    nc.compile()
    return nc


def _dma_table_out(nc, bounce, tab_sb):
    full = NB - 1
    rows = SHARD - full * BLK  # 106
    nc.sync.dma_start(
        out=bounce[0:full * BLK, :].rearrange("(b p) c -> p b c", p=BLK, b=full),
        in_=tab_sb[:, 0:full, :])
    nc.sync.dma_start(out=bounce[full * BLK:SHARD, :], in_=tab_sb[0:rows, full, :])


def _edge_phase(nc, tc, layer, table_full, idx_ext, msk_ext, ident,
                plan, a_s, a_d, bias, w2_t, tab_out, out_ext, b2_t,
                dbg_ext=None):
    """ELL-layout per-edge softmax-aggregate phase (see module docstring)."""
    if layer == 1:
        NH, CH, CC = H1, HID, D1       # 8 heads x 8 ch = 64
        asrc_sl = (64, 80)
        adst_sl = (80, 96)
    else:
        NH, CH, CC = 1, C2, C2         # 1 head x 40
        asrc_sl = (64, 66)
        adst_sl = (66, 68)
    NCOL = CC + NH
    sub = int(os.environ.get("K_EDGE_SUB", "4"))

    with tc.tile_pool(name=f"e{layer}", bufs=2) as pool, \
         tc.tile_pool(name=f"e{layer}_ps", bufs=2, space="PSUM") as psum, \
         tc.tile_pool(name=f"e{layer}_ps2", bufs=2, space="PSUM") as psum2:
        for blocks, goff, gch, nlo_ch in plan.groups:
            g_t = pool.tile([128, gch, TROW], bfl, tag="gath")
            nhi_ch = gch - nlo_ch
            idxg = pool.tile([128, gch * 8], i16, tag="idxg")
            nc.sync.dma_start(out=idxg[:, :],
                              in_=idx_ext[:, goff * 8:(goff + gch) * 8])
            mskg = pool.tile([128, gch], bfl, tag="mskg")
            nc.sync.dma_start(out=mskg[:, :], in_=msk_ext[:, goff:goff + gch])
            nc.gpsimd.dma_gather(
                out_ap=g_t[:, 0:nlo_ch, :], in_ap=table_full[0:SPLIT, :],
                idxs_ap=idxg[:, 0:nlo_ch * 8],
                num_idxs=nlo_ch * BLK, num_idxs_reg=nlo_ch * BLK,
                elem_size=TROW, single_packet=False)
            nc.gpsimd.dma_gather(
                out_ap=g_t[:, nlo_ch:gch, :], in_ap=table_full[SPLIT:N_NODES, :],
                idxs_ap=idxg[:, nlo_ch * 8:gch * 8],
                num_idxs=nhi_ch * BLK, num_idxs_reg=nhi_ch * BLK,
                elem_size=TROW, single_packet=False, queue_num=1)

            if sub < 2:
                continue
            for b in blocks:
                lo0, nbl = plan.lo_runs[b]
                hi0, nbh = plan.hi_runs[b]
                lo0 -= goff
                hi0 -= goff
                mlo = plan.mlo_ch[b] - goff
                mhi = plan.mhi_ch[b] - goff

                # a_dst[dst] per partition: blend member-lo / member-hi rows
                mf = pool.tile([128, 2], f32, tag="memmask")
                nc.vector.tensor_copy(out=mf[:, 0:1], in_=mskg[:, mlo:mlo + 1])
                nc.vector.tensor_copy(out=mf[:, 1:2], in_=mskg[:, mhi:mhi + 1])
                adst = pool.tile([128, NH], f32, tag="adst")
                tmp_hi = pool.tile([128, NH], f32, tag="adsth")
                nc.vector.tensor_scalar(
                    out=tmp_hi[:, :],
                    in0=g_t[:, mhi, adst_sl[0]:adst_sl[1]].bitcast(f32),
                    scalar1=mf[:, 1:2], scalar2=None,
                    op0=mybir.AluOpType.mult)
                nc.vector.scalar_tensor_tensor(
                    out=adst[:, :],
                    in0=g_t[:, mlo, adst_sl[0]:adst_sl[1]].bitcast(f32),
                    scalar=mf[:, 0:1], in1=tmp_hi[:, :],
                    op0=mybir.AluOpType.mult, op1=mybir.AluOpType.add)

                ps = psum.tile([128, NCOL], f32, tag="agg")
                r_ts = []
                for r0, rn in ((lo0, nbl), (hi0, nbh)):
                    tg = "h" if r0 == hi0 else "l"
                    e_t = pool.tile([128, rn, NH], f32, tag=f"elog{tg}")
                    nc.vector.tensor_tensor(
                        out=e_t[:, :, :],
                        in0=g_t[:, r0:r0 + rn,
                                asrc_sl[0]:asrc_sl[1]].bitcast(f32),
                        in1=adst[:, None, :].to_broadcast([128, rn, NH]),
                        op=mybir.AluOpType.add)
                    lr_t = pool.tile([128, rn, NH], f32, tag=f"lr{tg}")
                    nc.vector.scalar_tensor_tensor(
                        out=lr_t[:, :, :], in0=e_t[:, :, :], scalar=NEG_SLOPE,
                        in1=e_t[:, :, :], op0=mybir.AluOpType.mult,
                        op1=mybir.AluOpType.max)
                    exf = pool.tile([128, rn, NH], f32, tag=f"exf{tg}")
                    nc.scalar.activation(out=exf[:, :, :], in_=lr_t[:, :, :],
                                         func=mybir.ActivationFunctionType.Exp)
                    r_t = pool.tile([128, rn, NCOL], bfl, tag=f"rmat{tg}")
                    nc.vector.tensor_tensor(
                        out=r_t[:, :, CC:NCOL], in0=exf[:, :, :],
                        in1=mskg[:, r0:r0 + rn, None].to_broadcast([128, rn, NH]),
                        op=mybir.AluOpType.mult)
                    nc.vector.tensor_tensor(
                        out=r_t[:, :, 0:CC].rearrange("p g (h c) -> p g h c",
                                                      h=NH, c=CH),
                        in0=g_t[:, r0:r0 + rn, 0:CC].rearrange(
                            "p g (h c) -> p g h c", h=NH, c=CH),
                        in1=r_t[:, :, CC:NCOL, None].to_broadcast(
                            [128, rn, NH, CH]),
                        op=mybir.AluOpType.mult)
                    r_ts.append((r_t, rn))
                if sub < 3:
                    continue
                nmm = sum(rn for _, rn in r_ts)
                ji = 0
                for r_t, rn in r_ts:
                    for j in range(rn):
                        nc.tensor.matmul(ps[:, :], lhsT=ident[:, :],
                                         rhs=r_t[:, j, :], start=(ji == 0),
                                         stop=(ji == nmm - 1))
                        ji += 1

                den = pool.tile([128, NH], f32, tag="den")
                nc.vector.tensor_scalar(out=den[:, :], in0=ps[:, CC:NCOL],
                                        scalar1=1e-16, scalar2=None,
                                        op0=mybir.AluOpType.add)
                recip = pool.tile([128, NH], f32, tag="recip")
                nc.vector.reciprocal(out=recip[:, :], in_=den[:, :])
                o_t = pool.tile([128, CC], f32, tag="outb")
                nc.vector.tensor_tensor(
                    out=o_t[:, :].rearrange("p (h c) -> p h c", h=NH, c=CH),
                    in0=ps[:, 0:CC].rearrange("p (h c) -> p h c", h=NH, c=CH),
                    in1=recip[:, :, None].to_broadcast([128, NH, CH]),
                    op=mybir.AluOpType.mult)

                if sub < 4:
                    continue
                if layer == 1:
                    obt = pool.tile([128, CC], f32, tag="outbt")
                    nc.vector.tensor_tensor(out=obt[:, :], in0=o_t[:, :],
                                            in1=bias[:, :],
                                            op=mybir.AluOpType.add)
                    ob = pool.tile([128, CC], bfl, tag="outbf")
                    nc.vector.tensor_scalar(out=ob[:, :], in0=obt[:, :],
                                            scalar1=0.0, scalar2=None,
                                            op0=mybir.AluOpType.max)
                    if dbg_ext is not None:
                        dtt = pool.tile([128, NCOL], f32, tag="dbgt")
                        nc.vector.tensor_copy(out=dtt[:, :], in_=ps[:, :])
                        nc.sync.dma_start(
                            out=dbg_ext[b * BLK:(b + 1) * BLK, :],
                            in_=dtt[:, :])
                    tps = psum2.tile([D1, 128], bfl, tag="tp")
                    nc.tensor.transpose(tps[:, :], ob[:, :], ident[:, :])
                    h1T = pool.tile([D1, 128], bfl, tag="h1T")
                    nc.vector.tensor_copy(out=h1T[:, :], in_=tps[:, :])
                    h2ps = psum2.tile([128, C2], f32, tag="h2")
                    nc.tensor.matmul(h2ps[:, :], lhsT=h1T[:, :], rhs=w2_t[:, :],
                                     start=True, stop=True)
                    nc.scalar.activation(out=tab_out[:, b, 0:C2],
                                         in_=h2ps[:, :],
                                         func=mybir.ActivationFunctionType.Copy)
                    t1 = pool.tile([128, C2], f32, tag="t1")
                    nc.vector.tensor_tensor(out=t1[:, :], in0=h2ps[:, :],
                                            in1=a_s[:, :],
                                            op=mybir.AluOpType.mult)
                    nc.vector.tensor_reduce(
                        out=tab_out[:, b, 64:66].bitcast(f32), in_=t1[:, :],
                        axis=mybir.AxisListType.X, op=mybir.AluOpType.add)
                    t2 = pool.tile([128, C2], f32, tag="t2")
                    nc.vector.tensor_tensor(out=t2[:, :], in0=h2ps[:, :],
                                            in1=a_d[:, :],
                                            op=mybir.AluOpType.mult)
                    nc.vector.tensor_reduce(
                        out=tab_out[:, b, 66:68].bitcast(f32), in_=t2[:, :],
                        axis=mybir.AxisListType.X, op=mybir.AluOpType.add)
                else:
                    lg = pool.tile([128, C2], f32, tag="logits")
                    nc.vector.tensor_tensor(out=lg[:, :], in0=o_t[:, :],
                                            in1=b2_t[:, :],
                                            op=mybir.AluOpType.add)
                    negm = pool.tile([128, 1], f32, tag="negm")
                    nc.vector.tensor_reduce(out=negm[:, :], in_=lg[:, :],
                                            axis=mybir.AxisListType.X,
                                            op=mybir.AluOpType.max, negate=True)
                    ex = pool.tile([128, C2], f32, tag="sfex")
                    ssum = pool.tile([128, 1], f32, tag="ssum")
                    nc.scalar.activation(out=ex[:, :], in_=lg[:, :],
                                         func=mybir.ActivationFunctionType.Exp,
                                         bias=negm[:, :], accum_out=ssum[:, :])
                    lse = pool.tile([128, 1], f32, tag="lse")
                    nc.scalar.activation(out=lse[:, :], in_=ssum[:, :],
                                         func=mybir.ActivationFunctionType.Ln)
                    res = pool.tile([128, C2], f32, tag="res")
                    nc.vector.scalar_tensor_tensor(
                        out=res[:, :], in0=lg[:, :], scalar=negm[:, :],
                        in1=lse[:, :].to_broadcast([128, C2]),
                        op0=mybir.AluOpType.add, op1=mybir.AluOpType.subtract)
                    nc.sync.dma_start(out=out_ext[b * BLK:(b + 1) * BLK, :],
                                      in_=res[:, :])


def _host_inputs(x, W1, att_src1, att_dst1, b1, W2, att_src2, att_dst2, b2,
                 plan1, plan2):
    NPADROWS = NB * BLK
    w1r = np.ascontiguousarray(
        np.asarray(W1, np.float32).reshape(4, 128, D1).transpose(1, 0, 2)
    ).reshape(128, 4 * D1).astype(bf16)
    rep = lambda v, n: np.tile(np.asarray(v, np.float32).reshape(1, n),
                               (128, 1)).astype(np.float32)
    x32 = np.asarray(x, np.float32)

    in_maps = []
    for c in range(N_CORES):
        xs = x32[c * SHARD:(c + 1) * SHARD]
        xT = np.zeros((F_IN, NPADROWS), bf16)
        xT[:, :SHARD] = xs.T.astype(bf16)
        in_maps.append({
            "xT": xT,
            "w1r": w1r,
            "w2": np.asarray(W2, np.float32).astype(bf16),
            "a1srep": rep(att_src1, D1),
            "a1drep": rep(att_dst1, D1),
            "a2srep": rep(att_src2, C2),
            "a2drep": rep(att_dst2, C2),
            "b1rep": rep(b1, D1),
            "b2rep": rep(b2, C2),
            "idx1": plan1.idx_streams[c],
            "msk1": plan1.mask_streams[c],
            "idx2": plan2.idx_streams[c],
            "msk2": plan2.mask_streams[c],
        })
    return in_maps


def kernel_run(inputs, trace=False):
    """Build (cached), run, and return (out [50000,40] f32, exec_time_ns)."""
    edge_index = inputs["edge_index"]
    plan1, plan2 = _prep(edge_index)

    key = (tuple(plan1.nch_lo), tuple(plan1.nch_hi),
           tuple(plan2.nch_lo), tuple(plan2.nch_hi))
    if key not in _CACHE:
        _CACHE[key] = _build(plan1, plan2)
    nc = _CACHE[key]

    in_maps = _host_inputs(
        inputs["x"], inputs["W1"], inputs["att_src1"], inputs["att_dst1"],
        inputs["b1"], inputs["W2"], inputs["att_src2"], inputs["att_dst2"],
        inputs["b2"], plan1, plan2)

    if trace:
        _install_ntff_hook()
    res = run_bass_kernel_spmd(nc, in_maps, core_ids=list(range(N_CORES)),
                               trace=trace)
    # undo the block permutation (output rows are layer-2 block slots)
    out = np.zeros((N_NODES, C2), np.float32)
    for c in range(N_CORES):
        o = res.results[c]["out"]
        mem = plan2.perm[c]
        valid = mem >= 0
        out[c * SHARD + mem[valid]] = o[valid]
    return out, res.exec_time_ns


def kernel(**inputs):
    out, _ = kernel_run(inputs)
    return out



# revision 6
# speedup vs baseline: 1.9598x; 1.9598x over previous
"""GAT 2-layer GNN kernel for 8 Trainium2 NeuronCores (v2).

Strategy (graph/data parallel, per the sharding hint):
  - The 50000 dst nodes are dealt round-robin from a global in-degree sort
    into 8 cores x 6272 slots (49 blocks of 128), so every block holds
    near-equal degrees on all cores and ELL padding is ~3%.
  - Table rows (per node: [h bf16 | a_src bf16 | pad] in a 256B row) live in
    a chunk-major DRAM table; both layers share one slot assignment, so one
    int16 index stream serves both edge phases.  Indices are SIGNED offsets
    from a mid-table base row, so a single dma_gather run per block covers
    all 50192 rows (no lo/hi split).  Padding slots point at a row filled
    with -60000: exp maps it to 0, so no mask streams are needed.
  - Per super-group (blocks packed to <=96 chunks) one SWDGE gather call
    fetches all edge rows; calls round-robin over 4 SWDGE queues, which
    parallelizes Q7 descriptor generation (measured ~2.9 ns/edge vs ~8
    single-queue).  exp(leakyrelu(a_src+a_dst)) is computed group-wide and
    written back over the a_src bytes, making each chunk's matmul rhs
    [h*alpha | alpha] contiguous; identity-matmul PSUM accumulation then
    yields per-dst [numerator | denominator].
  - Layer-1 results feed h2 = relu(out1) @ W2 per block; the layer-2 table
    is AllGathered in 4 chunk-major slices that overlap the remaining
    layer-1 compute (same for the layer-1 table under phase A).
  - Layer 2 (1 head x 40) repeats the pipeline and fuses log_softmax; the
    host undoes the slot permutation.
"""

import os
import sys

sys.path.insert(0, "/opt/trn_rl_repo")

import numpy as np
import ml_dtypes

import concourse.bacc as bacc
import concourse.mybir as mybir
from concourse import tile
from concourse.bass_utils import run_bass_kernel_spmd
from concourse.masks import make_identity

bf16 = ml_dtypes.bfloat16

N_NODES = 50000
F_IN = 512
H1 = 8
HID = 8
D1 = H1 * HID  # 64
C2 = 40
N_CORES = 8
BLK = 128
NB = 49
SLOTS = NB * BLK  # 6272
REAL_ROWS = N_CORES * SLOTS  # 50176
PAD_ROWS = 16
T_ROWS = REAL_ROWS + PAD_ROWS  # 50192
IDX_BASE = T_ROWS - 32768  # 17424
PAD_IDX = REAL_ROWS - IDX_BASE  # 32752 (the -60000 row)
TROW = 128  # table row: 128 bf16 = 256 bytes
NEG_SLOPE = 0.2
NEG_FILL = -60000.0
GMAX = 96  # max chunks per gather super-group
BPC = [13, 13, 13, 10]  # AllGather blocks per chunk
NQ = 4  # SWDGE queues

f32 = mybir.dt.float32
bfl = mybir.dt.bfloat16
i16 = mybir.dt.int16

_CACHE = {}


def _install_ntff_hook():
    """Provide antenv.axon_hooks if the image lacks it, driving NTFF
    profiling via the injected libaxon_pjrt.so C ABI (see trn_boot)."""
    try:
        from antenv.axon_hooks import get_axon_ntff_profile_hook  # noqa: F401
        return
    except ImportError:
        pass
    import contextlib
    import ctypes
    import types

    so_path = "/opt/axon/libaxon_pjrt.so"
    try:
        lib = ctypes.CDLL(so_path)
    except OSError:
        return
    if not hasattr(lib, "axon_start_nrt_profile"):
        return
    lib.axon_start_nrt_profile.argtypes = [ctypes.POINTER(ctypes.c_int64),
                                           ctypes.c_size_t]
    lib.axon_start_nrt_profile.restype = ctypes.c_int64
    lib.axon_stop_nrt_profile.argtypes = [ctypes.c_char_p]
    lib.axon_stop_nrt_profile.restype = ctypes.c_int64

    @contextlib.contextmanager
    def _hook(output_dir, device_ids):
        import jax
        jax.devices()
        if device_ids:
            ids = (ctypes.c_int64 * len(device_ids))(*device_ids)
            rc = lib.axon_start_nrt_profile(ids, len(device_ids))
        else:
            rc = lib.axon_start_nrt_profile(None, 0)
        if rc != 0:
            raise RuntimeError(f"axon_start_nrt_profile rc={rc}")
        try:
            yield
        finally:
            n = lib.axon_stop_nrt_profile(str(output_dir).encode())
            print(f"ntff profile: {n} file(s) written to {output_dir}")

    import antenv
    mod = types.ModuleType("antenv.axon_hooks")
    mod.get_axon_ntff_profile_hook = lambda: _hook
    mod.set_axon_ntff_profile_hook = lambda h: None
    sys.modules["antenv.axon_hooks"] = mod
    antenv.axon_hooks = mod


class Plan:
    pass


def _chunk_major_rows():
    """row(core, slot) for the chunk-major table layout."""
    rows_k = np.array(BPC) * BLK
    chunk_base = np.concatenate([[0], np.cumsum(N_CORES * rows_k)[:-1]])
    sb0 = np.concatenate([[0], np.cumsum(BPC)[:-1]])
    cum_b = np.cumsum(BPC)
    return rows_k, chunk_base, sb0, cum_b


def _prep(edge_index):
    src = np.asarray(edge_index[0], dtype=np.int64)
    dst = np.asarray(edge_index[1], dtype=np.int64)
    loops = np.arange(N_NODES, dtype=np.int64)
    src = np.concatenate([src, loops])
    dst = np.concatenate([dst, loops])

    plan = Plan()
    ktot = np.bincount(dst, minlength=N_NODES)
    order = np.argsort(-ktot, kind="stable")
    core_of = np.empty(N_NODES, np.int64)
    slot_of = np.empty(N_NODES, np.int64)
    core_of[order] = np.arange(N_NODES) % N_CORES
    slot_of[order] = np.arange(N_NODES) // N_CORES

    rows_k, chunk_base, sb0, cum_b = _chunk_major_rows()
    blk_of_slot = slot_of // BLK
    k_of = np.searchsorted(cum_b, blk_of_slot, side="right")
    row_of = (chunk_base[k_of] + core_of * rows_k[k_of]
              + (slot_of - sb0[k_of] * BLK))

    # per-(core,slot) degree -> per-block chunk count
    kP = np.zeros((N_CORES, SLOTS), np.int64)
    kP[core_of, slot_of] = ktot
    nch = kP.reshape(N_CORES, NB, BLK).max(axis=(0, 2))

    # super-groups: pack consecutive blocks, <= GMAX chunks; +1 guard chunk
    groups = []  # (blocks, goff, gch_total, {b: local chunk offset})
    chunk_start = np.zeros(NB, np.int64)  # slab chunk index of block's run
    goff = 0
    b = 0
    while b < NB:
        blocks = [b]
        tot = int(nch[b])
        b += 1
        while b < NB and tot + int(nch[b]) <= GMAX:
            blocks.append(b)
            tot += int(nch[b])
            b += 1
        ch = goff
        for blk in blocks:
            chunk_start[blk] = ch
            ch += int(nch[blk])
        groups.append((blocks, goff, tot + 1))  # +1 guard chunk
        goff += tot + 1
    tch = goff

    # per-core index slabs
    idx_slabs = []
    for c in range(N_CORES):
        sel = core_of[dst] == c
        e_src = src[sel]
        e_slot = slot_of[dst[sel]]
        o = np.argsort(e_slot, kind="stable")
        e_src = e_src[o]
        e_slot = e_slot[o]
        pos = _running_count(e_slot)
        p = e_slot % BLK
        blk = e_slot // BLK
        slabpos = (chunk_start[blk] + pos) * BLK + p
        slab = np.full(tch * BLK, PAD_IDX, np.int16)
        slab[slabpos] = (row_of[e_src] - IDX_BASE).astype(np.int16)
        slab_w = np.tile(slab.reshape(tch * 8, 16).T, (8, 1)).copy()
        idx_slabs.append(slab_w)

    plan.nch = nch
    plan.groups = groups
    plan.tch = tch
    plan.chunk_start = chunk_start
    plan.idx_slabs = idx_slabs
    plan.core_of = core_of
    plan.slot_of = slot_of
    plan.order = order
    return plan


def _running_count(k):
    """pos[i] = number of j<i with k[j]==k[i]; k is sorted."""
    n = len(k)
    if n == 0:
        return np.zeros(0, np.int64)
    starts = np.r_[0, np.flatnonzero(np.diff(k)) + 1]
    run_id = np.zeros(n, np.int64)
    run_id[starts[1:]] = 1
    run_id = np.cumsum(run_id)
    return np.arange(n) - starts[run_id]


def _build(plan):
    nc = bacc.Bacc("TRN2", target_bir_lowering=False, debug=False,
                   num_devices=N_CORES, num_swdge_queues=NQ)

    xT_ext = nc.declare_dram_parameter("xT", [F_IN, SLOTS], bfl, isOutput=False)
    w1_ext = nc.declare_dram_parameter("w1r", [128, 4 * D1], bfl, isOutput=False)
    w2_ext = nc.declare_dram_parameter("w2", [D1, C2], bfl, isOutput=False)
    a1s_ext = nc.declare_dram_parameter("a1srep", [128, D1], f32, isOutput=False)
    a1d_ext = nc.declare_dram_parameter("a1drep", [128, D1], f32, isOutput=False)
    a2s_ext = nc.declare_dram_parameter("a2srep", [128, C2], f32, isOutput=False)
    a2d_ext = nc.declare_dram_parameter("a2drep", [128, C2], f32, isOutput=False)
    b1_ext = nc.declare_dram_parameter("b1rep", [128, D1], f32, isOutput=False)
    b2_ext = nc.declare_dram_parameter("b2rep", [128, C2], f32, isOutput=False)
    idx_ext = nc.declare_dram_parameter("idxs", [128, plan.tch * 8], i16,
                                        isOutput=False)
    out_ext = nc.declare_dram_parameter("out", [SLOTS, C2], f32, isOutput=True)

    t1_shard = nc.dram_tensor("t1_shard", [SLOTS, TROW], bfl)
    t1_full = nc.dram_tensor("t1_full", [T_ROWS, TROW], bfl, addr_space="Shared")
    t2_shard = nc.dram_tensor("t2_shard", [SLOTS, TROW], bfl)
    t2_full = nc.dram_tensor("t2_full", [T_ROWS, TROW], bfl, addr_space="Shared")

    rg = [list(range(N_CORES))]
    rows_k, chunk_base, sb0, _ = _chunk_major_rows()

    def ag_chunks(shard, full):
        """(shard_slice, full_slice) per AllGather chunk."""
        out = []
        for k in range(len(BPC)):
            s0 = sb0[k] * BLK
            s1 = s0 + rows_k[k]
            f0 = chunk_base[k]
            f1 = f0 + N_CORES * rows_k[k]
            out.append((shard[int(s0):int(s1), :], full[int(f0):int(f1), :]))
        return out

    ag1 = ag_chunks(t1_shard, t1_full)
    ag2 = ag_chunks(t2_shard, t2_full)
    # block index after which AG chunk k's rows are complete
    ag_after = np.cumsum(BPC) - 1  # [12, 25, 38, 48]

    with tile.TileContext(nc) as tc:
        with tc.tile_pool(name="const", bufs=1) as cpool:
            ident = cpool.tile([128, 128], bfl)
            make_identity(nc, ident[:, :])
            a1s_t = cpool.tile([128, D1], f32)
            nc.sync.dma_start(out=a1s_t[:, :], in_=a1s_ext[:, :])
            a1d_t = cpool.tile([128, D1], f32)
            nc.sync.dma_start(out=a1d_t[:, :], in_=a1d_ext[:, :])
            a2s_t = cpool.tile([128, C2], f32)
            nc.sync.dma_start(out=a2s_t[:, :], in_=a2s_ext[:, :])
            a2d_t = cpool.tile([128, C2], f32)
            nc.sync.dma_start(out=a2d_t[:, :], in_=a2d_ext[:, :])
            b1_t = cpool.tile([128, D1], f32)
            nc.sync.dma_start(out=b1_t[:, :], in_=b1_ext[:, :])
            b2_t = cpool.tile([128, C2], f32)
            nc.sync.dma_start(out=b2_t[:, :], in_=b2_ext[:, :])
            w2_t = cpool.tile([D1, C2], bfl)
            nc.sync.dma_start(out=w2_t[:, :], in_=w2_ext[:, :])
            idx_t = cpool.tile([128, plan.tch * 8], i16)
            nc.sync.dma_start(out=idx_t[:, :], in_=idx_ext[:, :])
            adst1 = cpool.tile([128, NB, H1], f32)
            adst2 = cpool.tile([128, NB, 1], f32)

            # -60000 pad rows, written locally into both tables
            neg_t = cpool.tile([PAD_ROWS, TROW], bfl)
            nc.vector.memset(neg_t[:, :], NEG_FILL)
            nc.sync.dma_start(out=t1_full[REAL_ROWS:T_ROWS, :], in_=neg_t[:, :])
            nc.sync.dma_start(out=t2_full[REAL_ROWS:T_ROWS, :], in_=neg_t[:, :])

            # ---------------- Phase A: h1 = x @ W1, a_src1/a_dst1 ----------
            with tc.tile_pool(name="phA", bufs=2) as apool, \
                 tc.tile_pool(name="phA_ps", bufs=2, space="PSUM") as apsum:
                w1_t = apool.tile([128, 4, D1], bfl, tag="w1")
                nc.sync.dma_start(out=w1_t[:, :, :], in_=w1_ext[:, :])
                xk = []
                for k in range(4):
                    xt = apool.tile([128, SLOTS], bfl, tag=f"xk{k}")
                    nc.sync.dma_start(out=xt[:, :],
                                      in_=xT_ext[k * 128:(k + 1) * 128, :])
                    xk.append(xt)
                agq1 = list(ag1)
                for b in range(NB):
                    hps = apsum.tile([128, D1], f32, tag="hps")
                    for k in range(4):
                        nc.tensor.matmul(
                            hps[:, :], lhsT=xk[k][:, b * BLK:(b + 1) * BLK],
                            rhs=w1_t[:, k, :], start=(k == 0), stop=(k == 3))
                    t1row = apool.tile([128, 96], bfl, tag="t1row")
                    nc.scalar.activation(out=t1row[:, 0:D1], in_=hps[:, :],
                                         func=mybir.ActivationFunctionType.Copy)
                    tmp = apool.tile([128, D1], f32, tag="atmp")
                    nc.vector.tensor_tensor(out=tmp[:, :], in0=hps[:, :],
                                            in1=a1s_t[:, :],
                                            op=mybir.AluOpType.mult)
                    nc.vector.tensor_reduce(
                        out=t1row[:, D1:D1 + 2 * H1].bitcast(f32),
                        in_=tmp[:, :].rearrange("p (h c) -> p h c", h=H1, c=HID),
                        axis=mybir.AxisListType.X, op=mybir.AluOpType.add)
                    tmp2 = apool.tile([128, D1], f32, tag="atmp2")
                    nc.vector.tensor_tensor(out=tmp2[:, :], in0=hps[:, :],
                                            in1=a1d_t[:, :],
                                            op=mybir.AluOpType.mult)
                    nc.vector.tensor_reduce(
                        out=adst1[:, b, :],
                        in_=tmp2[:, :].rearrange("p (h c) -> p h c", h=H1, c=HID),
                        axis=mybir.AxisListType.X, op=mybir.AluOpType.add)
                    nc.sync.dma_start(
                        out=t1_shard[b * BLK:(b + 1) * BLK, 0:96],
                        in_=t1row[:, :])
                    if agq1 and b >= ag_after[len(ag1) - len(agq1)] + 1:
                        ins_ap, outs_ap = agq1.pop(0)
                        nc.gpsimd.collective_compute(
                            "AllGather", mybir.AluOpType.bypass,
                            replica_groups=rg,
                            ins=[ins_ap.opt()], outs=[outs_ap.opt()])
                while agq1:
                    ins_ap, outs_ap = agq1.pop(0)
                    nc.gpsimd.collective_compute(
                        "AllGather", mybir.AluOpType.bypass, replica_groups=rg,
                        ins=[ins_ap.opt()], outs=[outs_ap.opt()])

            # gathers read below their declared in_ap slice (signed idxs), so
            # the AG->gather dependency must be a hard barrier
            tc.strict_bb_all_engine_barrier()

            _edge_phase(nc, tc, plan, layer=1, table_full=t1_full, idx_t=idx_t,
                        ident=ident, adst=adst1, bias=b1_t, w2_t=w2_t,
                        a2s_t=a2s_t, a2d_t=a2d_t, adst_next=adst2,
                        t_next_shard=t2_shard, ag_next=ag2, ag_after=ag_after,
                        rg=rg, b2_t=None, out_ext=None)

            tc.strict_bb_all_engine_barrier()

            _edge_phase(nc, tc, plan, layer=2, table_full=t2_full, idx_t=idx_t,
                        ident=ident, adst=adst2, bias=None, w2_t=None,
                        a2s_t=None, a2d_t=None, adst_next=None,
                        t_next_shard=None, ag_next=None, ag_after=None,
                        rg=None, b2_t=b2_t, out_ext=out_ext)

    nc.compile()
    return nc


def _edge_phase(nc, tc, plan, layer, table_full, idx_t, ident, adst, bias,
                w2_t, a2s_t, a2d_t, adst_next, t_next_shard, ag_next, ag_after,
                rg, b2_t, out_ext):
    if layer == 1:
        NH, CH, CC = H1, HID, D1   # 8 heads x 8 ch
    else:
        NH, CH, CC = 1, C2, C2     # 1 head x 40
    NCOL = CC + NH
    base = table_full[IDX_BASE:T_ROWS, :]
    agq = list(ag_next) if ag_next else []
    qrr = [0]

    with tc.tile_pool(name=f"e{layer}", bufs=3) as pool, \
         tc.tile_pool(name=f"e{layer}s", bufs=2) as spool, \
         tc.tile_pool(name=f"e{layer}_ps", bufs=2, space="PSUM") as psum, \
         tc.tile_pool(name=f"e{layer}_ps2", bufs=2, space="PSUM") as psum2:
        for blocks, goff, gch in plan.groups:
            g_t = pool.tile([128, gch, TROW], bfl, tag="gath")
            nc.gpsimd.dma_gather(
                out_ap=g_t[:, :, :], in_ap=base,
                idxs_ap=idx_t[:, goff * 8:(goff + gch) * 8],
                num_idxs=gch * BLK, num_idxs_reg=gch * BLK,
                elem_size=TROW, single_packet=False,
                queue_num=qrr[0] % NQ)
            qrr[0] += 1
            nreal = gch - 1  # last chunk is the guard

            # group-wide alpha: exp(leakyrelu(a_src + a_dst)) -> a_src slot
            adst_g = pool.tile([128, nreal, NH], f32, tag="adstg")
            for blk in blocks:
                r0 = int(plan.chunk_start[blk]) - goff
                nchb = int(plan.nch[blk])
                nc.vector.tensor_copy(
                    out=adst_g[:, r0:r0 + nchb, :],
                    in_=adst[:, blk, None, :].to_broadcast([128, nchb, NH]))
            e_t = pool.tile([128, nreal, NH], f32, tag="elog")
            nc.vector.tensor_tensor(
                out=e_t[:, :, :],
                in0=g_t[:, 0:nreal, CC:CC + 2 * NH].bitcast(f32),
                in1=adst_g[:, :, :], op=mybir.AluOpType.add)
            lr_t = pool.tile([128, nreal, NH], f32, tag="lrelu")
            nc.vector.scalar_tensor_tensor(
                out=lr_t[:, :, :], in0=e_t[:, :, :], scalar=NEG_SLOPE,
                in1=e_t[:, :, :], op0=mybir.AluOpType.mult,
                op1=mybir.AluOpType.max)
            nc.scalar.activation(out=g_t[:, 0:nreal, CC:NCOL], in_=lr_t[:, :, :],
                                 func=mybir.ActivationFunctionType.Exp)
            nc.vector.tensor_tensor(
                out=g_t[:, 0:nreal, 0:CC].rearrange(
                    "p g (h c) -> p g h c", h=NH, c=CH),
                in0=g_t[:, 0:nreal, 0:CC].rearrange(
                    "p g (h c) -> p g h c", h=NH, c=CH),
                in1=g_t[:, 0:nreal, CC:NCOL, None].to_broadcast(
                    [128, nreal, NH, CH]),
                op=mybir.AluOpType.mult)

            for blk in blocks:
                r0 = int(plan.chunk_start[blk]) - goff
                nchb = int(plan.nch[blk])
                ps = psum.tile([128, NCOL], f32, tag="agg")
                for j in range(nchb):
                    nc.tensor.matmul(ps[:, :], lhsT=ident[:, :],
                                     rhs=g_t[:, r0 + j, 0:NCOL],
                                     start=(j == 0), stop=(j == nchb - 1))

                recip = spool.tile([128, NH], f32, tag="recip")
                nc.vector.reciprocal(out=recip[:, :], in_=ps[:, CC:NCOL])
                o_t = spool.tile([128, CC], f32, tag="outb")
                nc.vector.tensor_tensor(
                    out=o_t[:, :].rearrange("p (h c) -> p h c", h=NH, c=CH),
                    in0=ps[:, 0:CC].rearrange("p (h c) -> p h c", h=NH, c=CH),
                    in1=recip[:, :, None].to_broadcast([128, NH, CH]),
                    op=mybir.AluOpType.mult)

                if layer == 1:
                    obt = spool.tile([128, CC], f32, tag="outbt")
                    nc.vector.tensor_tensor(out=obt[:, :], in0=o_t[:, :],
                                            in1=bias[:, :],
                                            op=mybir.AluOpType.add)
                    ob = spool.tile([128, CC], bfl, tag="outbf")
                    nc.vector.tensor_scalar(out=ob[:, :], in0=obt[:, :],
                                            scalar1=0.0, scalar2=None,
                                            op0=mybir.AluOpType.max)
                    tps = psum2.tile([D1, 128], bfl, tag="tp")
                    nc.tensor.transpose(tps[:, :], ob[:, :], ident[:, :])
                    h1T = spool.tile([D1, 128], bfl, tag="h1T")
                    nc.vector.tensor_copy(out=h1T[:, :], in_=tps[:, :])
                    h2ps = psum2.tile([128, C2], f32, tag="h2")
                    nc.tensor.matmul(h2ps[:, :], lhsT=h1T[:, :], rhs=w2_t[:, :],
                                     start=True, stop=True)
                    t2row = spool.tile([128, 48], bfl, tag="t2row")
                    nc.scalar.activation(out=t2row[:, 0:C2], in_=h2ps[:, :],
                                         func=mybir.ActivationFunctionType.Copy)
                    t1 = spool.tile([128, C2], f32, tag="t1")
                    nc.vector.tensor_tensor(out=t1[:, :], in0=h2ps[:, :],
                                            in1=a2s_t[:, :],
                                            op=mybir.AluOpType.mult)
                    nc.vector.tensor_reduce(
                        out=t2row[:, C2:C2 + 2].bitcast(f32), in_=t1[:, :],
                        axis=mybir.AxisListType.X, op=mybir.AluOpType.add)
                    t2 = spool.tile([128, C2], f32, tag="t2")
                    nc.vector.tensor_tensor(out=t2[:, :], in0=h2ps[:, :],
                                            in1=a2d_t[:, :],
                                            op=mybir.AluOpType.mult)
                    nc.vector.tensor_reduce(
                        out=adst_next[:, blk, :], in_=t2[:, :],
                        axis=mybir.AxisListType.X, op=mybir.AluOpType.add)
                    nc.sync.dma_start(
                        out=t_next_shard[blk * BLK:(blk + 1) * BLK, 0:48],
                        in_=t2row[:, :])
                else:
                    lg = spool.tile([128, C2], f32, tag="logits")
                    nc.vector.tensor_tensor(out=lg[:, :], in0=o_t[:, :],
                                            in1=b2_t[:, :],
                                            op=mybir.AluOpType.add)
                    negm = spool.tile([128, 1], f32, tag="negm")
                    nc.vector.tensor_reduce(out=negm[:, :], in_=lg[:, :],
                                            axis=mybir.AxisListType.X,
                                            op=mybir.AluOpType.max, negate=True)
                    ex = spool.tile([128, C2], f32, tag="sfex")
                    ssum = spool.tile([128, 1], f32, tag="ssum")
                    nc.scalar.activation(out=ex[:, :], in_=lg[:, :],
                                         func=mybir.ActivationFunctionType.Exp,
                                         bias=negm[:, :], accum_out=ssum[:, :])
                    lse = spool.tile([128, 1], f32, tag="lse")
                    nc.scalar.activation(out=lse[:, :], in_=ssum[:, :],
                                         func=mybir.ActivationFunctionType.Ln)
                    res = spool.tile([128, C2], f32, tag="res")
                    nc.vector.scalar_tensor_tensor(
                        out=res[:, :], in0=lg[:, :], scalar=negm[:, :],
                        in1=lse[:, :].to_broadcast([128, C2]),
                        op0=mybir.AluOpType.add, op1=mybir.AluOpType.subtract)
                    nc.sync.dma_start(out=out_ext[blk * BLK:(blk + 1) * BLK, :],
                                      in_=res[:, :])

                if agq and blk >= ag_after[len(ag_next) - len(agq)] + 1:
                    ins_ap, outs_ap = agq.pop(0)
                    nc.gpsimd.collective_compute(
                        "AllGather", mybir.AluOpType.bypass, replica_groups=rg,
                        ins=[ins_ap.opt()], outs=[outs_ap.opt()])
        while agq:
            ins_ap, outs_ap = agq.pop(0)
            nc.gpsimd.collective_compute(
                "AllGather", mybir.AluOpType.bypass, replica_groups=rg,
                ins=[ins_ap.opt()], outs=[outs_ap.opt()])


def _host_inputs(x, W1, att_src1, att_dst1, b1, W2, att_src2, att_dst2, b2,
                 plan):
    w1r = np.ascontiguousarray(
        np.asarray(W1, np.float32).reshape(4, 128, D1).transpose(1, 0, 2)
    ).reshape(128, 4 * D1).astype(bf16)
    rep = lambda v, n: np.tile(np.asarray(v, np.float32).reshape(1, n),
                               (128, 1)).astype(np.float32)
    x32 = np.asarray(x, np.float32)

    # xT per core: column s = x[node at (c, s)].T
    order = plan.order
    in_maps = []
    for c in range(N_CORES):
        ranks = np.arange(SLOTS) * N_CORES + c
        valid = ranks < N_NODES
        nodes = order[ranks[valid]]
        xT = np.zeros((F_IN, SLOTS), bf16)
        xT[:, valid] = x32[nodes].T.astype(bf16)
        in_maps.append({
            "xT": xT,
            "w1r": w1r,
            "w2": np.asarray(W2, np.float32).astype(bf16),
            "a1srep": rep(att_src1, D1),
            "a1drep": rep(att_dst1, D1),
            "a2srep": rep(att_src2, C2),
            "a2drep": rep(att_dst2, C2),
            "b1rep": rep(b1, D1),
            "b2rep": rep(b2, C2),
            "idxs": plan.idx_slabs[c],
        })
    return in_maps


def kernel_run(inputs, trace=False):
    """Build (cached), run, and return (out [50000,40] f32, exec_time_ns)."""
    edge_index = inputs["edge_index"]
    plan = _prep(edge_index)

    key = tuple(plan.nch)
    if key not in _CACHE:
        _CACHE[key] = _build(plan)
    nc = _CACHE[key]

    in_maps = _host_inputs(
        inputs["x"], inputs["W1"], inputs["att_src1"], inputs["att_dst1"],
        inputs["b1"], inputs["W2"], inputs["att_src2"], inputs["att_dst2"],
        inputs["b2"], plan)

    if trace:
        _install_ntff_hook()
    res = run_bass_kernel_spmd(nc, in_maps, core_ids=list(range(N_CORES)),
                               trace=trace)
    # undo the slot permutation
    out = np.zeros((N_NODES, C2), np.float32)
    order = plan.order
    for c in range(N_CORES):
        o = res.results[c]["out"]
        ranks = np.arange(SLOTS) * N_CORES + c
        valid = ranks < N_NODES
        out[order[ranks[valid]]] = o[valid]
    return out, res.exec_time_ns


def kernel(**inputs):
    out, _ = kernel_run(inputs)
    return out


# revision 11
# speedup vs baseline: 1.9879x; 1.0143x over previous
"""GAT 2-layer GNN kernel for 8 Trainium2 NeuronCores (v2).

Strategy (graph/data parallel, per the sharding hint):
  - The 50000 dst nodes are dealt round-robin from a global in-degree sort
    into 8 cores x 6272 slots (49 blocks of 128), so every block holds
    near-equal degrees on all cores and ELL padding is ~3%.
  - Table rows (per node: [h bf16 | a_src bf16 | pad] in a 256B row) live in
    a chunk-major DRAM table; both layers share one slot assignment, so one
    int16 index stream serves both edge phases.  Indices are SIGNED offsets
    from a mid-table base row, so a single dma_gather run per block covers
    all 50192 rows (no lo/hi split).  Padding slots point at a row filled
    with -60000: exp maps it to 0, so no mask streams are needed.
  - Per super-group (blocks packed to <=96 chunks) one SWDGE gather call
    fetches all edge rows; calls round-robin over 4 SWDGE queues, which
    parallelizes Q7 descriptor generation (measured ~2.9 ns/edge vs ~8
    single-queue).  exp(leakyrelu(a_src+a_dst)) is computed group-wide and
    written back over the a_src bytes, making each chunk's matmul rhs
    [h*alpha | alpha] contiguous; identity-matmul PSUM accumulation then
    yields per-dst [numerator | denominator].
  - Layer-1 results feed h2 = relu(out1) @ W2 per block; the layer-2 table
    is AllGathered in 4 chunk-major slices that overlap the remaining
    layer-1 compute (same for the layer-1 table under phase A).
  - Layer 2 (1 head x 40) repeats the pipeline and fuses log_softmax; the
    host undoes the slot permutation.
"""

import os
import sys

sys.path.insert(0, "/opt/trn_rl_repo")

import numpy as np
import ml_dtypes

import concourse.bacc as bacc
import concourse.mybir as mybir
from concourse import tile
from concourse.bass_utils import run_bass_kernel_spmd
from concourse.masks import make_identity

bf16 = ml_dtypes.bfloat16

N_NODES = 50000
F_IN = 512
H1 = 8
HID = 8
D1 = H1 * HID  # 64
C2 = 40
N_CORES = 8
BLK = 128
NB = 49
SLOTS = NB * BLK  # 6272
REAL_ROWS = N_CORES * SLOTS  # 50176
PAD_ROWS = 16
T_ROWS = REAL_ROWS + PAD_ROWS  # 50192
IDX_BASE = T_ROWS - 32768  # 17424
PAD_IDX = REAL_ROWS - IDX_BASE  # 32752 (the -60000 row)
TROW = 128  # table row: 128 bf16 = 256 bytes
NEG_SLOPE = 0.2
NEG_FILL = -60000.0
GMAX = 88  # max chunks per gather super-group
BPC = [13, 13, 13, 10]  # AllGather blocks per chunk
AG_LAG = 6  # blocks of emission lag before an AG chunk (head-of-line)
NQ = 4  # SWDGE queues

f32 = mybir.dt.float32
bfl = mybir.dt.bfloat16
i16 = mybir.dt.int16

_CACHE = {}


def _install_ntff_hook():
    """Provide antenv.axon_hooks if the image lacks it, driving NTFF
    profiling via the injected libaxon_pjrt.so C ABI (see trn_boot)."""
    try:
        from antenv.axon_hooks import get_axon_ntff_profile_hook  # noqa: F401
        return
    except ImportError:
        pass
    import contextlib
    import ctypes
    import types

    so_path = "/opt/axon/libaxon_pjrt.so"
    try:
        lib = ctypes.CDLL(so_path)
    except OSError:
        return
    if not hasattr(lib, "axon_start_nrt_profile"):
        return
    lib.axon_start_nrt_profile.argtypes = [ctypes.POINTER(ctypes.c_int64),
                                           ctypes.c_size_t]
    lib.axon_start_nrt_profile.restype = ctypes.c_int64
    lib.axon_stop_nrt_profile.argtypes = [ctypes.c_char_p]
    lib.axon_stop_nrt_profile.restype = ctypes.c_int64

    @contextlib.contextmanager
    def _hook(output_dir, device_ids):
        import jax
        jax.devices()
        if device_ids:
            ids = (ctypes.c_int64 * len(device_ids))(*device_ids)
            rc = lib.axon_start_nrt_profile(ids, len(device_ids))
        else:
            rc = lib.axon_start_nrt_profile(None, 0)
        if rc != 0:
            raise RuntimeError(f"axon_start_nrt_profile rc={rc}")
        try:
            yield
        finally:
            n = lib.axon_stop_nrt_profile(str(output_dir).encode())
            print(f"ntff profile: {n} file(s) written to {output_dir}")

    import antenv
    mod = types.ModuleType("antenv.axon_hooks")
    mod.get_axon_ntff_profile_hook = lambda: _hook
    mod.set_axon_ntff_profile_hook = lambda h: None
    sys.modules["antenv.axon_hooks"] = mod
    antenv.axon_hooks = mod


class Plan:
    pass


def _chunk_major_rows():
    """row(core, slot) for the chunk-major table layout."""
    rows_k = np.array(BPC) * BLK
    chunk_base = np.concatenate([[0], np.cumsum(N_CORES * rows_k)[:-1]])
    sb0 = np.concatenate([[0], np.cumsum(BPC)[:-1]])
    cum_b = np.cumsum(BPC)
    return rows_k, chunk_base, sb0, cum_b


def _prep(edge_index):
    src = np.asarray(edge_index[0], dtype=np.int64)
    dst = np.asarray(edge_index[1], dtype=np.int64)
    loops = np.arange(N_NODES, dtype=np.int64)
    src = np.concatenate([src, loops])
    dst = np.concatenate([dst, loops])

    plan = Plan()
    ktot = np.bincount(dst, minlength=N_NODES)
    order = np.argsort(-ktot, kind="stable")
    core_of = np.empty(N_NODES, np.int64)
    slot_of = np.empty(N_NODES, np.int64)
    core_of[order] = np.arange(N_NODES) % N_CORES
    slot_of[order] = np.arange(N_NODES) // N_CORES

    rows_k, chunk_base, sb0, cum_b = _chunk_major_rows()
    blk_of_slot = slot_of // BLK
    k_of = np.searchsorted(cum_b, blk_of_slot, side="right")
    row_of = (chunk_base[k_of] + core_of * rows_k[k_of]
              + (slot_of - sb0[k_of] * BLK))

    # per-(core,slot) degree -> per-block chunk count
    kP = np.zeros((N_CORES, SLOTS), np.int64)
    kP[core_of, slot_of] = ktot
    nch = kP.reshape(N_CORES, NB, BLK).max(axis=(0, 2))

    # super-groups: pack consecutive blocks, <= GMAX chunks; +1 guard chunk
    groups = []  # (blocks, goff, gch_total, {b: local chunk offset})
    chunk_start = np.zeros(NB, np.int64)  # slab chunk index of block's run
    goff = 0
    b = 0
    while b < NB:
        blocks = [b]
        tot = int(nch[b])
        b += 1
        while b < NB and tot + int(nch[b]) <= GMAX:
            blocks.append(b)
            tot += int(nch[b])
            b += 1
        ch = goff
        for blk in blocks:
            chunk_start[blk] = ch
            ch += int(nch[blk])
        groups.append((blocks, goff, tot + 1))  # +1 guard chunk
        goff += tot + 1
    tch = goff

    # per-core index slabs
    idx_slabs = []
    for c in range(N_CORES):
        sel = core_of[dst] == c
        e_src = src[sel]
        e_slot = slot_of[dst[sel]]
        o = np.argsort(e_slot, kind="stable")
        e_src = e_src[o]
        e_slot = e_slot[o]
        pos = _running_count(e_slot)
        p = e_slot % BLK
        blk = e_slot // BLK
        slabpos = (chunk_start[blk] + pos) * BLK + p
        slab = np.full(tch * BLK, PAD_IDX, np.int16)
        slab[slabpos] = (row_of[e_src] - IDX_BASE).astype(np.int16)
        slab_w = np.tile(slab.reshape(tch * 8, 16).T, (8, 1)).copy()
        idx_slabs.append(slab_w)

    plan.nch = nch
    plan.groups = groups
    plan.tch = tch
    plan.chunk_start = chunk_start
    plan.idx_slabs = idx_slabs
    plan.core_of = core_of
    plan.slot_of = slot_of
    plan.order = order
    return plan


def _running_count(k):
    """pos[i] = number of j<i with k[j]==k[i]; k is sorted."""
    n = len(k)
    if n == 0:
        return np.zeros(0, np.int64)
    starts = np.r_[0, np.flatnonzero(np.diff(k)) + 1]
    run_id = np.zeros(n, np.int64)
    run_id[starts[1:]] = 1
    run_id = np.cumsum(run_id)
    return np.arange(n) - starts[run_id]


def _build(plan):
    nc = bacc.Bacc("TRN2", target_bir_lowering=False, debug=False,
                   num_devices=N_CORES, num_swdge_queues=NQ)

    xT_ext = nc.declare_dram_parameter("xT", [F_IN, SLOTS], bfl, isOutput=False)
    w1_ext = nc.declare_dram_parameter("w1r", [128, 4 * D1], bfl, isOutput=False)
    w2_ext = nc.declare_dram_parameter("w2", [D1, C2], bfl, isOutput=False)
    a1s_ext = nc.declare_dram_parameter("a1srep", [128, D1], f32, isOutput=False)
    a1d_ext = nc.declare_dram_parameter("a1drep", [128, D1], f32, isOutput=False)
    a2s_ext = nc.declare_dram_parameter("a2srep", [128, C2], f32, isOutput=False)
    a2d_ext = nc.declare_dram_parameter("a2drep", [128, C2], f32, isOutput=False)
    b1_ext = nc.declare_dram_parameter("b1rep", [128, D1], f32, isOutput=False)
    b2_ext = nc.declare_dram_parameter("b2rep", [128, C2], f32, isOutput=False)
    idx_ext = nc.declare_dram_parameter("idxs", [128, plan.tch * 8], i16,
                                        isOutput=False)
    out_ext = nc.declare_dram_parameter("out", [SLOTS, C2], f32, isOutput=True)

    t1_shard = nc.dram_tensor("t1_shard", [SLOTS, TROW], bfl)
    t1_full = nc.dram_tensor("t1_full", [T_ROWS, TROW], bfl, addr_space="Shared")
    t2_shard = nc.dram_tensor("t2_shard", [SLOTS, TROW], bfl)
    t2_full = nc.dram_tensor("t2_full", [T_ROWS, TROW], bfl, addr_space="Shared")

    rg = [list(range(N_CORES))]
    rows_k, chunk_base, sb0, _ = _chunk_major_rows()

    def ag_chunks(shard, full):
        """(shard_slice, full_slice) per AllGather chunk."""
        out = []
        for k in range(len(BPC)):
            s0 = sb0[k] * BLK
            s1 = s0 + rows_k[k]
            f0 = chunk_base[k]
            f1 = f0 + N_CORES * rows_k[k]
            out.append((shard[int(s0):int(s1), :], full[int(f0):int(f1), :]))
        return out

    ag1 = ag_chunks(t1_shard, t1_full)
    ag2 = ag_chunks(t2_shard, t2_full)
    # block index after which AG chunk k's rows are complete
    ag_after = np.cumsum(BPC) - 1  # [12, 25, 38, 48]

    with tile.TileContext(nc) as tc:
        with tc.tile_pool(name="const", bufs=1) as cpool:
            ident = cpool.tile([128, 128], bfl)
            make_identity(nc, ident[:, :])
            a1s_t = cpool.tile([128, D1], f32)
            nc.sync.dma_start(out=a1s_t[:, :], in_=a1s_ext[:, :])
            a1d_t = cpool.tile([128, D1], f32)
            nc.sync.dma_start(out=a1d_t[:, :], in_=a1d_ext[:, :])
            a2s_t = cpool.tile([128, C2], f32)
            nc.sync.dma_start(out=a2s_t[:, :], in_=a2s_ext[:, :])
            a2d_t = cpool.tile([128, C2], f32)
            nc.sync.dma_start(out=a2d_t[:, :], in_=a2d_ext[:, :])
            b1_t = cpool.tile([128, D1], f32)
            nc.sync.dma_start(out=b1_t[:, :], in_=b1_ext[:, :])
            b2_t = cpool.tile([128, C2], f32)
            nc.sync.dma_start(out=b2_t[:, :], in_=b2_ext[:, :])
            w2_t = cpool.tile([D1, C2], bfl)
            nc.sync.dma_start(out=w2_t[:, :], in_=w2_ext[:, :])
            idx_t = cpool.tile([128, plan.tch * 8], i16)
            nc.sync.dma_start(out=idx_t[:, :], in_=idx_ext[:, :])
            adst1 = cpool.tile([128, NB, H1], f32)
            adst2 = cpool.tile([128, NB, 1], f32)

            # -60000 pad rows, written locally into both tables
            neg_t = cpool.tile([PAD_ROWS, TROW], bfl)
            nc.vector.memset(neg_t[:, :], NEG_FILL)
            nc.sync.dma_start(out=t1_full[REAL_ROWS:T_ROWS, :], in_=neg_t[:, :])
            nc.sync.dma_start(out=t2_full[REAL_ROWS:T_ROWS, :], in_=neg_t[:, :])

            # ---------------- Phase A: h1 = x @ W1, a_src1/a_dst1 ----------
            with tc.tile_pool(name="phA", bufs=2) as apool, \
                 tc.tile_pool(name="phA_ps", bufs=2, space="PSUM") as apsum:
                w1_t = apool.tile([128, 4, D1], bfl, tag="w1")
                nc.sync.dma_start(out=w1_t[:, :, :], in_=w1_ext[:, :])
                xk = []
                for k in range(4):
                    xt = apool.tile([128, SLOTS], bfl, tag=f"xk{k}")
                    nc.sync.dma_start(out=xt[:, :],
                                      in_=xT_ext[k * 128:(k + 1) * 128, :])
                    xk.append(xt)
                agq1 = list(ag1)
                for b in range(NB):
                    hps = apsum.tile([128, D1], f32, tag="hps")
                    for k in range(4):
                        nc.tensor.matmul(
                            hps[:, :], lhsT=xk[k][:, b * BLK:(b + 1) * BLK],
                            rhs=w1_t[:, k, :], start=(k == 0), stop=(k == 3))
                    t1row = apool.tile([128, 96], bfl, tag="t1row")
                    nc.scalar.activation(out=t1row[:, 0:D1], in_=hps[:, :],
                                         func=mybir.ActivationFunctionType.Copy)
                    tmp = apool.tile([128, D1], f32, tag="atmp")
                    nc.vector.tensor_tensor(out=tmp[:, :], in0=hps[:, :],
                                            in1=a1s_t[:, :],
                                            op=mybir.AluOpType.mult)
                    nc.vector.tensor_reduce(
                        out=t1row[:, D1:D1 + 2 * H1].bitcast(f32),
                        in_=tmp[:, :].rearrange("p (h c) -> p h c", h=H1, c=HID),
                        axis=mybir.AxisListType.X, op=mybir.AluOpType.add)
                    tmp2 = apool.tile([128, D1], f32, tag="atmp2")
                    nc.vector.tensor_tensor(out=tmp2[:, :], in0=hps[:, :],
                                            in1=a1d_t[:, :],
                                            op=mybir.AluOpType.mult)
                    nc.vector.tensor_reduce(
                        out=adst1[:, b, :],
                        in_=tmp2[:, :].rearrange("p (h c) -> p h c", h=H1, c=HID),
                        axis=mybir.AxisListType.X, op=mybir.AluOpType.add)
                    nc.sync.dma_start(
                        out=t1_shard[b * BLK:(b + 1) * BLK, 0:96],
                        in_=t1row[:, :])
                    if agq1 and b >= ag_after[len(ag1) - len(agq1)] + AG_LAG:
                        ins_ap, outs_ap = agq1.pop(0)
                        nc.gpsimd.collective_compute(
                            "AllGather", mybir.AluOpType.bypass,
                            replica_groups=rg,
                            ins=[ins_ap.opt()], outs=[outs_ap.opt()])
                while agq1:
                    ins_ap, outs_ap = agq1.pop(0)
                    nc.gpsimd.collective_compute(
                        "AllGather", mybir.AluOpType.bypass, replica_groups=rg,
                        ins=[ins_ap.opt()], outs=[outs_ap.opt()])

            # gathers read below their declared in_ap slice (signed idxs), so
            # the AG->gather dependency must be a hard barrier
            tc.strict_bb_all_engine_barrier()

            _edge_phase(nc, tc, plan, layer=1, table_full=t1_full, idx_t=idx_t,
                        ident=ident, adst=adst1, bias=b1_t, w2_t=w2_t,
                        a2s_t=a2s_t, a2d_t=a2d_t, adst_next=adst2,
                        t_next_shard=t2_shard, ag_next=ag2, ag_after=ag_after,
                        rg=rg, b2_t=None, out_ext=None)

            tc.strict_bb_all_engine_barrier()

            _edge_phase(nc, tc, plan, layer=2, table_full=t2_full, idx_t=idx_t,
                        ident=ident, adst=adst2, bias=None, w2_t=None,
                        a2s_t=None, a2d_t=None, adst_next=None,
                        t_next_shard=None, ag_next=None, ag_after=None,
                        rg=None, b2_t=b2_t, out_ext=out_ext)

    nc.compile()
    return nc


def _edge_phase(nc, tc, plan, layer, table_full, idx_t, ident, adst, bias,
                w2_t, a2s_t, a2d_t, adst_next, t_next_shard, ag_next, ag_after,
                rg, b2_t, out_ext):
    if layer == 1:
        NH, CH, CC = H1, HID, D1   # 8 heads x 8 ch
    else:
        NH, CH, CC = 1, C2, C2     # 1 head x 40
    NCOL = CC + NH
    base = table_full[IDX_BASE:T_ROWS, :]
    agq = list(ag_next) if ag_next else []
    qrr = [0]
    ACT = mybir.ActivationFunctionType

    with tc.tile_pool(name=f"e{layer}", bufs=4) as pool, \
         tc.tile_pool(name=f"e{layer}t", bufs=3) as tpool, \
         tc.tile_pool(name=f"e{layer}s", bufs=2) as spool, \
         tc.tile_pool(name=f"e{layer}_ps", bufs=2, space="PSUM") as psum, \
         tc.tile_pool(name=f"e{layer}_ps2", bufs=2, space="PSUM") as psum2:
        for blocks, goff, gch in plan.groups:
            nblk = len(blocks)
            g_t = pool.tile([128, gch, TROW], bfl, tag="gath")
            nc.gpsimd.dma_gather(
                out_ap=g_t[:, :, :], in_ap=base,
                idxs_ap=idx_t[:, goff * 8:(goff + gch) * 8],
                num_idxs=gch * BLK, num_idxs_reg=gch * BLK,
                elem_size=TROW, single_packet=False,
                queue_num=qrr[0] % NQ)
            qrr[0] += 1
            nreal = gch - 1  # last chunk is the guard

            # group-wide alpha: exp(leakyrelu(a_src + a_dst)) -> a_src slot
            adst_g = tpool.tile([128, nreal, NH], f32, tag="adstg")
            for blk in blocks:
                r0 = int(plan.chunk_start[blk]) - goff
                nchb = int(plan.nch[blk])
                nc.vector.tensor_copy(
                    out=adst_g[:, r0:r0 + nchb, :],
                    in_=adst[:, blk, None, :].to_broadcast([128, nchb, NH]))
            e_t = tpool.tile([128, nreal, NH], f32, tag="elog")
            nc.vector.tensor_tensor(
                out=e_t[:, :, :],
                in0=g_t[:, 0:nreal, CC:CC + 2 * NH].bitcast(f32),
                in1=adst_g[:, :, :], op=mybir.AluOpType.add)
            lr_t = tpool.tile([128, nreal, NH], f32, tag="lrelu")
            nc.vector.scalar_tensor_tensor(
                out=lr_t[:, :, :], in0=e_t[:, :, :], scalar=NEG_SLOPE,
                in1=e_t[:, :, :], op0=mybir.AluOpType.mult,
                op1=mybir.AluOpType.max)
            nc.scalar.activation(out=g_t[:, 0:nreal, CC:CC + NH],
                                 in_=lr_t[:, :, :], func=ACT.Exp)
            nc.vector.tensor_tensor(
                out=g_t[:, 0:nreal, 0:CC].rearrange(
                    "p g (h c) -> p g h c", h=NH, c=CH),
                in0=g_t[:, 0:nreal, 0:CC].rearrange(
                    "p g (h c) -> p g h c", h=NH, c=CH),
                in1=g_t[:, 0:nreal, CC:CC + NH, None].to_broadcast(
                    [128, nreal, NH, CH]),
                op=mybir.AluOpType.mult)

            for blk in blocks:
                r0 = int(plan.chunk_start[blk]) - goff
                nchb = int(plan.nch[blk])
                ps = psum.tile([128, NCOL], f32, tag="agg")
                for j in range(nchb):
                    nc.tensor.matmul(ps[:, :], lhsT=ident[:, :],
                                     rhs=g_t[:, r0 + j, 0:NCOL],
                                     start=(j == 0), stop=(j == nchb - 1))

                recip = spool.tile([128, NH], f32, tag="recip")
                nc.vector.reciprocal(out=recip[:, :], in_=ps[:, CC:NCOL])
                o_t = spool.tile([128, CC], f32, tag="outb")
                nc.vector.tensor_tensor(
                    out=o_t[:, :].rearrange("p (h c) -> p h c", h=NH, c=CH),
                    in0=ps[:, 0:CC].rearrange("p (h c) -> p h c", h=NH, c=CH),
                    in1=recip[:, :, None].to_broadcast([128, NH, CH]),
                    op=mybir.AluOpType.mult)

                if layer == 1:
                    obt = spool.tile([128, CC], f32, tag="outbt")
                    nc.vector.tensor_tensor(out=obt[:, :], in0=o_t[:, :],
                                            in1=bias[:, :],
                                            op=mybir.AluOpType.add)
                    ob = spool.tile([128, CC], bfl, tag="outbf")
                    nc.vector.tensor_scalar(out=ob[:, :], in0=obt[:, :],
                                            scalar1=0.0, scalar2=None,
                                            op0=mybir.AluOpType.max)
                    tps = psum2.tile([D1, 128], bfl, tag="tp")
                    nc.tensor.transpose(tps[:, :], ob[:, :], ident[:, :])
                    h1T = spool.tile([D1, 128], bfl, tag="h1T")
                    nc.vector.tensor_copy(out=h1T[:, :], in_=tps[:, :])
                    h2ps = psum2.tile([128, C2], f32, tag="h2")
                    nc.tensor.matmul(h2ps[:, :], lhsT=h1T[:, :], rhs=w2_t[:, :],
                                     start=True, stop=True)
                    t2row = spool.tile([128, 48], bfl, tag="t2row")
                    nc.scalar.activation(out=t2row[:, 0:C2], in_=h2ps[:, :],
                                         func=ACT.Copy)
                    t1 = spool.tile([128, C2], f32, tag="t1")
                    nc.vector.tensor_tensor(out=t1[:, :], in0=h2ps[:, :],
                                            in1=a2s_t[:, :],
                                            op=mybir.AluOpType.mult)
                    nc.vector.tensor_reduce(
                        out=t2row[:, C2:C2 + 2].bitcast(f32), in_=t1[:, :],
                        axis=mybir.AxisListType.X, op=mybir.AluOpType.add)
                    t2 = spool.tile([128, C2], f32, tag="t2")
                    nc.vector.tensor_tensor(out=t2[:, :], in0=h2ps[:, :],
                                            in1=a2d_t[:, :],
                                            op=mybir.AluOpType.mult)
                    nc.vector.tensor_reduce(
                        out=adst_next[:, blk, :], in_=t2[:, :],
                        axis=mybir.AxisListType.X, op=mybir.AluOpType.add)
                    nc.sync.dma_start(
                        out=t_next_shard[blk * BLK:(blk + 1) * BLK, 0:48],
                        in_=t2row[:, :])
                else:
                    lg = spool.tile([128, C2], f32, tag="logits")
                    nc.vector.tensor_tensor(out=lg[:, :], in0=o_t[:, :],
                                            in1=b2_t[:, :],
                                            op=mybir.AluOpType.add)
                    negm = spool.tile([128, 1], f32, tag="negm")
                    nc.vector.tensor_reduce(out=negm[:, :], in_=lg[:, :],
                                            axis=mybir.AxisListType.X,
                                            op=mybir.AluOpType.max, negate=True)
                    ex = spool.tile([128, C2], f32, tag="sfex")
                    ssum = spool.tile([128, 1], f32, tag="ssum")
                    nc.scalar.activation(out=ex[:, :], in_=lg[:, :],
                                         func=ACT.Exp,
                                         bias=negm[:, :], accum_out=ssum[:, :])
                    lse = spool.tile([128, 1], f32, tag="lse")
                    nc.scalar.activation(out=lse[:, :], in_=ssum[:, :],
                                         func=ACT.Ln)
                    res = spool.tile([128, C2], f32, tag="res")
                    nc.vector.scalar_tensor_tensor(
                        out=res[:, :], in0=lg[:, :], scalar=negm[:, :],
                        in1=lse[:, :].to_broadcast([128, C2]),
                        op0=mybir.AluOpType.add, op1=mybir.AluOpType.subtract)
                    nc.sync.dma_start(out=out_ext[blk * BLK:(blk + 1) * BLK, :],
                                      in_=res[:, :])

            last_blk = blocks[-1]
            while agq and last_blk >= ag_after[len(ag_next) - len(agq)] + AG_LAG:
                ins_ap, outs_ap = agq.pop(0)
                nc.gpsimd.collective_compute(
                    "AllGather", mybir.AluOpType.bypass, replica_groups=rg,
                    ins=[ins_ap.opt()], outs=[outs_ap.opt()])
        while agq:
            ins_ap, outs_ap = agq.pop(0)
            nc.gpsimd.collective_compute(
                "AllGather", mybir.AluOpType.bypass, replica_groups=rg,
                ins=[ins_ap.opt()], outs=[outs_ap.opt()])


def _host_inputs(x, W1, att_src1, att_dst1, b1, W2, att_src2, att_dst2, b2,
                 plan):
    w1r = np.ascontiguousarray(
        np.asarray(W1, np.float32).reshape(4, 128, D1).transpose(1, 0, 2)
    ).reshape(128, 4 * D1).astype(bf16)
    rep = lambda v, n: np.tile(np.asarray(v, np.float32).reshape(1, n),
                               (128, 1)).astype(np.float32)
    x32 = np.asarray(x, np.float32)

    # xT per core: column s = x[node at (c, s)].T
    order = plan.order
    in_maps = []
    for c in range(N_CORES):
        ranks = np.arange(SLOTS) * N_CORES + c
        valid = ranks < N_NODES
        nodes = order[ranks[valid]]
        xT = np.zeros((F_IN, SLOTS), bf16)
        xT[:, valid] = x32[nodes].T.astype(bf16)
        in_maps.append({
            "xT": xT,
            "w1r": w1r,
            "w2": np.asarray(W2, np.float32).astype(bf16),
            "a1srep": rep(att_src1, D1),
            "a1drep": rep(att_dst1, D1),
            "a2srep": rep(att_src2, C2),
            "a2drep": rep(att_dst2, C2),
            "b1rep": rep(b1, D1),
            "b2rep": rep(b2, C2),
            "idxs": plan.idx_slabs[c],
        })
    return in_maps


def kernel_run(inputs, trace=False):
    """Build (cached), run, and return (out [50000,40] f32, exec_time_ns)."""
    edge_index = inputs["edge_index"]
    plan = _prep(edge_index)

    key = tuple(plan.nch)
    if key not in _CACHE:
        _CACHE[key] = _build(plan)
    nc = _CACHE[key]

    in_maps = _host_inputs(
        inputs["x"], inputs["W1"], inputs["att_src1"], inputs["att_dst1"],
        inputs["b1"], inputs["W2"], inputs["att_src2"], inputs["att_dst2"],
        inputs["b2"], plan)

    if trace:
        _install_ntff_hook()
    res = run_bass_kernel_spmd(nc, in_maps, core_ids=list(range(N_CORES)),
                               trace=trace)
    # undo the slot permutation
    out = np.zeros((N_NODES, C2), np.float32)
    order = plan.order
    for c in range(N_CORES):
        o = res.results[c]["out"]
        ranks = np.arange(SLOTS) * N_CORES + c
        valid = ranks < N_NODES
        out[order[ranks[valid]]] = o[valid]
    return out, res.exec_time_ns


def kernel(**inputs):
    out, _ = kernel_run(inputs)
    return out


# revision 12
# speedup vs baseline: 2.7518x; 1.3843x over previous
"""GAT 2-layer GNN kernel for 8 Trainium2 NeuronCores (v2).

Strategy (graph/data parallel, per the sharding hint):
  - The 50000 dst nodes are dealt round-robin from a global in-degree sort
    into 8 cores x 6272 slots (49 blocks of 128), so every block holds
    near-equal degrees on all cores and ELL padding is ~3%.
  - Table rows (per node: [h bf16 | a_src bf16 | pad] in a 256B row) live in
    a chunk-major DRAM table; both layers share one slot assignment, so one
    int16 index stream serves both edge phases.  Indices are SIGNED offsets
    from a mid-table base row, so a single dma_gather run per block covers
    all 50192 rows (no lo/hi split).  Padding slots point at a row filled
    with -60000: exp maps it to 0, so no mask streams are needed.
  - Per super-group (blocks packed to <=96 chunks) one SWDGE gather call
    fetches all edge rows; calls round-robin over 4 SWDGE queues, which
    parallelizes Q7 descriptor generation (measured ~2.9 ns/edge vs ~8
    single-queue).  exp(leakyrelu(a_src+a_dst)) is computed group-wide and
    written back over the a_src bytes, making each chunk's matmul rhs
    [h*alpha | alpha] contiguous; identity-matmul PSUM accumulation then
    yields per-dst [numerator | denominator].
  - Layer-1 results feed h2 = relu(out1) @ W2 per block; the layer-2 table
    is AllGathered in 4 chunk-major slices that overlap the remaining
    layer-1 compute (same for the layer-1 table under phase A).
  - Layer 2 (1 head x 40) repeats the pipeline and fuses log_softmax; the
    host undoes the slot permutation.
"""

import os
import sys

sys.path.insert(0, "/opt/trn_rl_repo")

import numpy as np
import ml_dtypes

import concourse.bacc as bacc
import concourse.mybir as mybir
from concourse import tile
from concourse.bass_utils import run_bass_kernel_spmd
from concourse.masks import make_identity

bf16 = ml_dtypes.bfloat16

N_NODES = 50000
F_IN = 512
H1 = 8
HID = 8
D1 = H1 * HID  # 64
C2 = 40
N_CORES = 8
BLK = 128
NB = 49
SLOTS = NB * BLK  # 6272
REAL_ROWS = N_CORES * SLOTS  # 50176
PAD_ROWS = 16
T_ROWS = REAL_ROWS + PAD_ROWS  # 50192
IDX_BASE = T_ROWS - 32768  # 17424
PAD_IDX = REAL_ROWS - IDX_BASE  # 32752 (the -60000 row)
TROW = 128  # table row: 128 bf16 = 256 bytes
NEG_SLOPE = 0.2
NEG_FILL = -60000.0
GMAX = 88  # max chunks per gather super-group
BPC = [13, 13, 13, 10]  # AllGather blocks per chunk
AG_LAG = 6  # blocks of emission lag before an AG chunk (head-of-line)
NQ = 4  # SWDGE queues

f32 = mybir.dt.float32
bfl = mybir.dt.bfloat16
i16 = mybir.dt.int16

_CACHE = {}


def _install_ntff_hook():
    """Provide antenv.axon_hooks if the image lacks it, driving NTFF
    profiling via the injected libaxon_pjrt.so C ABI (see trn_boot)."""
    try:
        from antenv.axon_hooks import get_axon_ntff_profile_hook  # noqa: F401
        return
    except ImportError:
        pass
    import contextlib
    import ctypes
    import types

    so_path = "/opt/axon/libaxon_pjrt.so"
    try:
        lib = ctypes.CDLL(so_path)
    except OSError:
        return
    if not hasattr(lib, "axon_start_nrt_profile"):
        return
    lib.axon_start_nrt_profile.argtypes = [ctypes.POINTER(ctypes.c_int64),
                                           ctypes.c_size_t]
    lib.axon_start_nrt_profile.restype = ctypes.c_int64
    lib.axon_stop_nrt_profile.argtypes = [ctypes.c_char_p]
    lib.axon_stop_nrt_profile.restype = ctypes.c_int64

    @contextlib.contextmanager
    def _hook(output_dir, device_ids):
        import jax
        jax.devices()
        if device_ids:
            ids = (ctypes.c_int64 * len(device_ids))(*device_ids)
            rc = lib.axon_start_nrt_profile(ids, len(device_ids))
        else:
            rc = lib.axon_start_nrt_profile(None, 0)
        if rc != 0:
            raise RuntimeError(f"axon_start_nrt_profile rc={rc}")
        try:
            yield
        finally:
            n = lib.axon_stop_nrt_profile(str(output_dir).encode())
            print(f"ntff profile: {n} file(s) written to {output_dir}")

    import antenv
    mod = types.ModuleType("antenv.axon_hooks")
    mod.get_axon_ntff_profile_hook = lambda: _hook
    mod.set_axon_ntff_profile_hook = lambda h: None
    sys.modules["antenv.axon_hooks"] = mod
    antenv.axon_hooks = mod


class Plan:
    pass


def _chunk_major_rows():
    """row(core, slot) for the chunk-major table layout."""
    rows_k = np.array(BPC) * BLK
    chunk_base = np.concatenate([[0], np.cumsum(N_CORES * rows_k)[:-1]])
    sb0 = np.concatenate([[0], np.cumsum(BPC)[:-1]])
    cum_b = np.cumsum(BPC)
    return rows_k, chunk_base, sb0, cum_b


def _prep(edge_index):
    src = np.asarray(edge_index[0], dtype=np.int64)
    dst = np.asarray(edge_index[1], dtype=np.int64)
    loops = np.arange(N_NODES, dtype=np.int64)
    src = np.concatenate([src, loops])
    dst = np.concatenate([dst, loops])

    plan = Plan()
    ktot = np.bincount(dst, minlength=N_NODES)
    order = np.argsort(-ktot, kind="stable")
    core_of = np.empty(N_NODES, np.int64)
    slot_of = np.empty(N_NODES, np.int64)
    core_of[order] = np.arange(N_NODES) % N_CORES
    slot_of[order] = np.arange(N_NODES) // N_CORES

    rows_k, chunk_base, sb0, cum_b = _chunk_major_rows()
    blk_of_slot = slot_of // BLK
    k_of = np.searchsorted(cum_b, blk_of_slot, side="right")
    row_of = (chunk_base[k_of] + core_of * rows_k[k_of]
              + (slot_of - sb0[k_of] * BLK))

    # per-(core,slot) degree -> per-block chunk count
    kP = np.zeros((N_CORES, SLOTS), np.int64)
    kP[core_of, slot_of] = ktot
    nch = kP.reshape(N_CORES, NB, BLK).max(axis=(0, 2))

    # super-groups: pack consecutive blocks, <= GMAX chunks; +1 guard chunk
    groups = []  # (blocks, goff, gch_total, {b: local chunk offset})
    chunk_start = np.zeros(NB, np.int64)  # slab chunk index of block's run
    goff = 0
    b = 0
    while b < NB:
        blocks = [b]
        tot = int(nch[b])
        b += 1
        while b < NB and tot + int(nch[b]) <= GMAX:
            blocks.append(b)
            tot += int(nch[b])
            b += 1
        ch = goff
        for blk in blocks:
            chunk_start[blk] = ch
            ch += int(nch[blk])
        groups.append((blocks, goff, tot + 1))  # +1 guard chunk
        goff += tot + 1
    tch = goff

    # per-core index slabs
    idx_slabs = []
    for c in range(N_CORES):
        sel = core_of[dst] == c
        e_src = src[sel]
        e_slot = slot_of[dst[sel]]
        o = np.argsort(e_slot, kind="stable")
        e_src = e_src[o]
        e_slot = e_slot[o]
        pos = _running_count(e_slot)
        p = e_slot % BLK
        blk = e_slot // BLK
        slabpos = (chunk_start[blk] + pos) * BLK + p
        slab = np.full(tch * BLK, PAD_IDX, np.int16)
        slab[slabpos] = (row_of[e_src] - IDX_BASE).astype(np.int16)
        slab_w = np.tile(slab.reshape(tch * 8, 16).T, (8, 1)).copy()
        idx_slabs.append(slab_w)

    plan.nch = nch
    plan.groups = groups
    plan.tch = tch
    plan.chunk_start = chunk_start
    plan.idx_slabs = idx_slabs
    plan.core_of = core_of
    plan.slot_of = slot_of
    plan.order = order
    return plan


def _running_count(k):
    """pos[i] = number of j<i with k[j]==k[i]; k is sorted."""
    n = len(k)
    if n == 0:
        return np.zeros(0, np.int64)
    starts = np.r_[0, np.flatnonzero(np.diff(k)) + 1]
    run_id = np.zeros(n, np.int64)
    run_id[starts[1:]] = 1
    run_id = np.cumsum(run_id)
    return np.arange(n) - starts[run_id]


def _build(plan):
    nc = bacc.Bacc("TRN2", target_bir_lowering=False, debug=False,
                   num_devices=N_CORES, num_swdge_queues=NQ)

    xT_ext = nc.declare_dram_parameter("xT", [F_IN, SLOTS], bfl, isOutput=False)
    w1_ext = nc.declare_dram_parameter("w1r", [128, 4 * D1], bfl, isOutput=False)
    w2_ext = nc.declare_dram_parameter("w2", [D1, C2], bfl, isOutput=False)
    a1s_ext = nc.declare_dram_parameter("a1srep", [128, D1], f32, isOutput=False)
    a1d_ext = nc.declare_dram_parameter("a1drep", [128, D1], f32, isOutput=False)
    a2s_ext = nc.declare_dram_parameter("a2srep", [128, C2], f32, isOutput=False)
    a2d_ext = nc.declare_dram_parameter("a2drep", [128, C2], f32, isOutput=False)
    b1_ext = nc.declare_dram_parameter("b1rep", [128, D1], f32, isOutput=False)
    b2_ext = nc.declare_dram_parameter("b2rep", [128, C2], f32, isOutput=False)
    idx_ext = nc.declare_dram_parameter("idxs", [128, plan.tch * 8], i16,
                                        isOutput=False)
    out_ext = nc.declare_dram_parameter("out", [SLOTS, C2], f32, isOutput=True)

    t1_shard = nc.dram_tensor("t1_shard", [SLOTS, TROW], bfl)
    t1_full = nc.dram_tensor("t1_full", [T_ROWS, TROW], bfl, addr_space="Shared")
    t2_shard = nc.dram_tensor("t2_shard", [SLOTS, TROW], bfl)
    t2_full = nc.dram_tensor("t2_full", [T_ROWS, TROW], bfl, addr_space="Shared")

    rg = [list(range(N_CORES))]
    rows_k, chunk_base, sb0, _ = _chunk_major_rows()

    def ag_chunks(shard, full):
        """(shard_slice, full_slice) per AllGather chunk."""
        out = []
        for k in range(len(BPC)):
            s0 = sb0[k] * BLK
            s1 = s0 + rows_k[k]
            f0 = chunk_base[k]
            f1 = f0 + N_CORES * rows_k[k]
            out.append((shard[int(s0):int(s1), :], full[int(f0):int(f1), :]))
        return out

    ag1 = ag_chunks(t1_shard, t1_full)
    ag2 = ag_chunks(t2_shard, t2_full)
    # block index after which AG chunk k's rows are complete
    ag_after = np.cumsum(BPC) - 1  # [12, 25, 38, 48]

    with tile.TileContext(nc) as tc:
        with tc.tile_pool(name="const", bufs=1) as cpool:
            ident = cpool.tile([128, 128], bfl)
            make_identity(nc, ident[:, :])
            a1s_t = cpool.tile([128, D1], f32)
            nc.sync.dma_start(out=a1s_t[:, :], in_=a1s_ext[:, :])
            a1d_t = cpool.tile([128, D1], f32)
            nc.sync.dma_start(out=a1d_t[:, :], in_=a1d_ext[:, :])
            a2s_t = cpool.tile([128, C2], f32)
            nc.sync.dma_start(out=a2s_t[:, :], in_=a2s_ext[:, :])
            a2d_t = cpool.tile([128, C2], f32)
            nc.sync.dma_start(out=a2d_t[:, :], in_=a2d_ext[:, :])
            b1_t = cpool.tile([128, D1], f32)
            nc.sync.dma_start(out=b1_t[:, :], in_=b1_ext[:, :])
            b2_t = cpool.tile([128, C2], f32)
            nc.sync.dma_start(out=b2_t[:, :], in_=b2_ext[:, :])
            w2_t = cpool.tile([D1, C2], bfl)
            nc.sync.dma_start(out=w2_t[:, :], in_=w2_ext[:, :])
            idx_t = cpool.tile([128, plan.tch * 8], i16)
            nc.sync.dma_start(out=idx_t[:, :], in_=idx_ext[:, :])
            adst1 = cpool.tile([128, NB, H1], f32)
            adst2 = cpool.tile([128, NB, 1], f32)

            # -60000 pad rows, written locally into both tables
            neg_t = cpool.tile([PAD_ROWS, TROW], bfl)
            nc.vector.memset(neg_t[:, :], NEG_FILL)
            nc.sync.dma_start(out=t1_full[REAL_ROWS:T_ROWS, :], in_=neg_t[:, :])
            nc.sync.dma_start(out=t2_full[REAL_ROWS:T_ROWS, :], in_=neg_t[:, :])

            # ---------------- Phase A: h1 = x @ W1, a_src1/a_dst1 ----------
            with tc.tile_pool(name="phA", bufs=2) as apool, \
                 tc.tile_pool(name="phA_ps", bufs=2, space="PSUM") as apsum:
                w1_t = apool.tile([128, 4, D1], bfl, tag="w1")
                nc.sync.dma_start(out=w1_t[:, :, :], in_=w1_ext[:, :])
                xk = []
                for k in range(4):
                    xt = apool.tile([128, SLOTS], bfl, tag=f"xk{k}")
                    nc.sync.dma_start(out=xt[:, :],
                                      in_=xT_ext[k * 128:(k + 1) * 128, :])
                    xk.append(xt)
                agq1 = list(ag1)
                for b in range(NB):
                    hps = apsum.tile([128, D1], f32, tag="hps")
                    for k in range(4):
                        nc.tensor.matmul(
                            hps[:, :], lhsT=xk[k][:, b * BLK:(b + 1) * BLK],
                            rhs=w1_t[:, k, :], start=(k == 0), stop=(k == 3))
                    t1row = apool.tile([128, 96], bfl, tag="t1row")
                    nc.scalar.activation(out=t1row[:, 0:D1], in_=hps[:, :],
                                         func=mybir.ActivationFunctionType.Copy)
                    tmp = apool.tile([128, D1], f32, tag="atmp")
                    nc.vector.tensor_tensor(out=tmp[:, :], in0=hps[:, :],
                                            in1=a1s_t[:, :],
                                            op=mybir.AluOpType.mult)
                    nc.vector.tensor_reduce(
                        out=t1row[:, D1:D1 + 2 * H1].bitcast(f32),
                        in_=tmp[:, :].rearrange("p (h c) -> p h c", h=H1, c=HID),
                        axis=mybir.AxisListType.X, op=mybir.AluOpType.add)
                    tmp2 = apool.tile([128, D1], f32, tag="atmp2")
                    nc.vector.tensor_tensor(out=tmp2[:, :], in0=hps[:, :],
                                            in1=a1d_t[:, :],
                                            op=mybir.AluOpType.mult)
                    nc.vector.tensor_reduce(
                        out=adst1[:, b, :],
                        in_=tmp2[:, :].rearrange("p (h c) -> p h c", h=H1, c=HID),
                        axis=mybir.AxisListType.X, op=mybir.AluOpType.add)
                    nc.sync.dma_start(
                        out=t1_shard[b * BLK:(b + 1) * BLK, 0:96],
                        in_=t1row[:, :])
                    if agq1 and b >= ag_after[len(ag1) - len(agq1)] + AG_LAG:
                        ins_ap, outs_ap = agq1.pop(0)
                        nc.gpsimd.collective_compute(
                            "AllGather", mybir.AluOpType.bypass,
                            replica_groups=rg,
                            ins=[ins_ap.opt()], outs=[outs_ap.opt()])
                while agq1:
                    ins_ap, outs_ap = agq1.pop(0)
                    nc.gpsimd.collective_compute(
                        "AllGather", mybir.AluOpType.bypass, replica_groups=rg,
                        ins=[ins_ap.opt()], outs=[outs_ap.opt()])

            # gathers read below their declared in_ap slice (signed idxs), so
            # the AG->gather dependency must be a hard barrier
            tc.strict_bb_all_engine_barrier()

            _edge_phase(nc, tc, plan, layer=1, table_full=t1_full, idx_t=idx_t,
                        ident=ident, adst=adst1, bias=b1_t, w2_t=w2_t,
                        a2s_t=a2s_t, a2d_t=a2d_t, adst_next=adst2,
                        t_next_shard=t2_shard, ag_next=ag2, ag_after=ag_after,
                        rg=rg, b2_t=None, out_ext=None)

            tc.strict_bb_all_engine_barrier()

            _edge_phase(nc, tc, plan, layer=2, table_full=t2_full, idx_t=idx_t,
                        ident=ident, adst=adst2, bias=None, w2_t=None,
                        a2s_t=None, a2d_t=None, adst_next=None,
                        t_next_shard=None, ag_next=None, ag_after=None,
                        rg=None, b2_t=b2_t, out_ext=out_ext)

    nc.compile()
    return nc


def _edge_phase(nc, tc, plan, layer, table_full, idx_t, ident, adst, bias,
                w2_t, a2s_t, a2d_t, adst_next, t_next_shard, ag_next, ag_after,
                rg, b2_t, out_ext):
    if layer == 1:
        NH, CH, CC = H1, HID, D1   # 8 heads x 8 ch
    else:
        NH, CH, CC = 1, C2, C2     # 1 head x 40
    NCOL = CC + NH
    base = table_full[IDX_BASE:T_ROWS, :]
    agq = list(ag_next) if ag_next else []
    qrr = [0]
    ACT = mybir.ActivationFunctionType

    with tc.tile_pool(name=f"e{layer}", bufs=4) as pool, \
         tc.tile_pool(name=f"e{layer}t", bufs=3) as tpool, \
         tc.tile_pool(name=f"e{layer}s", bufs=2) as spool, \
         tc.tile_pool(name=f"e{layer}_ps", bufs=2, space="PSUM") as psum, \
         tc.tile_pool(name=f"e{layer}_ps2", bufs=2, space="PSUM") as psum2:
        for blocks, goff, gch in plan.groups:
            nblk = len(blocks)
            g_t = pool.tile([128, gch, TROW], bfl, tag="gath")
            nc.gpsimd.dma_gather(
                out_ap=g_t[:, :, :], in_ap=base,
                idxs_ap=idx_t[:, goff * 8:(goff + gch) * 8],
                num_idxs=gch * BLK, num_idxs_reg=gch * BLK,
                elem_size=TROW, single_packet=False,
                queue_num=qrr[0] % NQ)
            qrr[0] += 1
            nreal = gch - 1  # last chunk is the guard

            # group-wide alpha: exp(leakyrelu(a_src + a_dst)) -> a_src slot
            e_t = tpool.tile([128, nreal, NH], f32, tag="elog")
            for blk in blocks:
                r0 = int(plan.chunk_start[blk]) - goff
                nchb = int(plan.nch[blk])
                nc.vector.tensor_tensor(
                    out=e_t[:, r0:r0 + nchb, :],
                    in0=g_t[:, r0:r0 + nchb, CC:CC + 2 * NH].bitcast(f32),
                    in1=adst[:, blk, None, :].to_broadcast([128, nchb, NH]),
                    op=mybir.AluOpType.add)
            lr_t = tpool.tile([128, nreal, NH], f32, tag="lrelu")
            nc.vector.scalar_tensor_tensor(
                out=lr_t[:, :, :], in0=e_t[:, :, :], scalar=NEG_SLOPE,
                in1=e_t[:, :, :], op0=mybir.AluOpType.mult,
                op1=mybir.AluOpType.max)
            nc.scalar.activation(out=g_t[:, 0:nreal, CC:CC + NH],
                                 in_=lr_t[:, :, :], func=ACT.Exp)
            nc.vector.tensor_tensor(
                out=g_t[:, 0:nreal, 0:CC].rearrange(
                    "p g (h c) -> p g h c", h=NH, c=CH),
                in0=g_t[:, 0:nreal, 0:CC].rearrange(
                    "p g (h c) -> p g h c", h=NH, c=CH),
                in1=g_t[:, 0:nreal, CC:CC + NH, None].to_broadcast(
                    [128, nreal, NH, CH]),
                op=mybir.AluOpType.mult)

            for blk in blocks:
                r0 = int(plan.chunk_start[blk]) - goff
                nchb = int(plan.nch[blk])
                ps = psum.tile([128, NCOL], f32, tag="agg")
                for j in range(nchb):
                    nc.tensor.matmul(ps[:, :], lhsT=ident[:, :],
                                     rhs=g_t[:, r0 + j, 0:NCOL],
                                     start=(j == 0), stop=(j == nchb - 1))

                recip = spool.tile([128, NH], f32, tag="recip")
                nc.vector.reciprocal(out=recip[:, :], in_=ps[:, CC:NCOL])
                o_t = spool.tile([128, CC], f32, tag="outb")
                nc.vector.tensor_tensor(
                    out=o_t[:, :].rearrange("p (h c) -> p h c", h=NH, c=CH),
                    in0=ps[:, 0:CC].rearrange("p (h c) -> p h c", h=NH, c=CH),
                    in1=recip[:, :, None].to_broadcast([128, NH, CH]),
                    op=mybir.AluOpType.mult)

                if layer == 1:
                    obt = spool.tile([128, CC], f32, tag="outbt")
                    nc.vector.tensor_tensor(out=obt[:, :], in0=o_t[:, :],
                                            in1=bias[:, :],
                                            op=mybir.AluOpType.add)
                    ob = spool.tile([128, CC], bfl, tag="outbf")
                    nc.vector.tensor_scalar(out=ob[:, :], in0=obt[:, :],
                                            scalar1=0.0, scalar2=None,
                                            op0=mybir.AluOpType.max)
                    tps = psum2.tile([D1, 128], bfl, tag="tp")
                    nc.tensor.transpose(tps[:, :], ob[:, :], ident[:, :])
                    h1T = spool.tile([D1, 128], bfl, tag="h1T")
                    nc.vector.tensor_copy(out=h1T[:, :], in_=tps[:, :])
                    h2ps = psum2.tile([128, C2], f32, tag="h2")
                    nc.tensor.matmul(h2ps[:, :], lhsT=h1T[:, :], rhs=w2_t[:, :],
                                     start=True, stop=True)
                    t2row = spool.tile([128, 48], bfl, tag="t2row")
                    nc.scalar.activation(out=t2row[:, 0:C2], in_=h2ps[:, :],
                                         func=ACT.Copy)
                    t1 = spool.tile([128, C2], f32, tag="t1")
                    nc.vector.tensor_tensor(out=t1[:, :], in0=h2ps[:, :],
                                            in1=a2s_t[:, :],
                                            op=mybir.AluOpType.mult)
                    nc.vector.tensor_reduce(
                        out=t2row[:, C2:C2 + 2].bitcast(f32), in_=t1[:, :],
                        axis=mybir.AxisListType.X, op=mybir.AluOpType.add)
                    t2 = spool.tile([128, C2], f32, tag="t2")
                    nc.vector.tensor_tensor(out=t2[:, :], in0=h2ps[:, :],
                                            in1=a2d_t[:, :],
                                            op=mybir.AluOpType.mult)
                    nc.vector.tensor_reduce(
                        out=adst_next[:, blk, :], in_=t2[:, :],
                        axis=mybir.AxisListType.X, op=mybir.AluOpType.add)
                    nc.sync.dma_start(
                        out=t_next_shard[blk * BLK:(blk + 1) * BLK, 0:48],
                        in_=t2row[:, :])
                else:
                    lg = spool.tile([128, C2], f32, tag="logits")
                    nc.vector.tensor_tensor(out=lg[:, :], in0=o_t[:, :],
                                            in1=b2_t[:, :],
                                            op=mybir.AluOpType.add)
                    negm = spool.tile([128, 1], f32, tag="negm")
                    nc.vector.tensor_reduce(out=negm[:, :], in_=lg[:, :],
                                            axis=mybir.AxisListType.X,
                                            op=mybir.AluOpType.max, negate=True)
                    ex = spool.tile([128, C2], f32, tag="sfex")
                    ssum = spool.tile([128, 1], f32, tag="ssum")
                    nc.scalar.activation(out=ex[:, :], in_=lg[:, :],
                                         func=ACT.Exp,
                                         bias=negm[:, :], accum_out=ssum[:, :])
                    lse = spool.tile([128, 1], f32, tag="lse")
                    nc.scalar.activation(out=lse[:, :], in_=ssum[:, :],
                                         func=ACT.Ln)
                    res = spool.tile([128, C2], f32, tag="res")
                    nc.vector.scalar_tensor_tensor(
                        out=res[:, :], in0=lg[:, :], scalar=negm[:, :],
                        in1=lse[:, :].to_broadcast([128, C2]),
                        op0=mybir.AluOpType.add, op1=mybir.AluOpType.subtract)
                    nc.sync.dma_start(out=out_ext[blk * BLK:(blk + 1) * BLK, :],
                                      in_=res[:, :])

            last_blk = blocks[-1]
            while agq and last_blk >= ag_after[len(ag_next) - len(agq)] + AG_LAG:
                ins_ap, outs_ap = agq.pop(0)
                nc.gpsimd.collective_compute(
                    "AllGather", mybir.AluOpType.bypass, replica_groups=rg,
                    ins=[ins_ap.opt()], outs=[outs_ap.opt()])
        while agq:
            ins_ap, outs_ap = agq.pop(0)
            nc.gpsimd.collective_compute(
                "AllGather", mybir.AluOpType.bypass, replica_groups=rg,
                ins=[ins_ap.opt()], outs=[outs_ap.opt()])


def _host_inputs(x, W1, att_src1, att_dst1, b1, W2, att_src2, att_dst2, b2,
                 plan):
    w1r = np.ascontiguousarray(
        np.asarray(W1, np.float32).reshape(4, 128, D1).transpose(1, 0, 2)
    ).reshape(128, 4 * D1).astype(bf16)
    rep = lambda v, n: np.tile(np.asarray(v, np.float32).reshape(1, n),
                               (128, 1)).astype(np.float32)
    x32 = np.asarray(x, np.float32)

    # xT per core: column s = x[node at (c, s)].T
    order = plan.order
    in_maps = []
    for c in range(N_CORES):
        ranks = np.arange(SLOTS) * N_CORES + c
        valid = ranks < N_NODES
        nodes = order[ranks[valid]]
        xT = np.zeros((F_IN, SLOTS), bf16)
        xT[:, valid] = x32[nodes].T.astype(bf16)
        in_maps.append({
            "xT": xT,
            "w1r": w1r,
            "w2": np.asarray(W2, np.float32).astype(bf16),
            "a1srep": rep(att_src1, D1),
            "a1drep": rep(att_dst1, D1),
            "a2srep": rep(att_src2, C2),
            "a2drep": rep(att_dst2, C2),
            "b1rep": rep(b1, D1),
            "b2rep": rep(b2, C2),
            "idxs": plan.idx_slabs[c],
        })
    return in_maps


def kernel_run(inputs, trace=False):
    """Build (cached), run, and return (out [50000,40] f32, exec_time_ns)."""
    edge_index = inputs["edge_index"]
    plan = _prep(edge_index)

    key = tuple(plan.nch)
    if key not in _CACHE:
        _CACHE[key] = _build(plan)
    nc = _CACHE[key]

    in_maps = _host_inputs(
        inputs["x"], inputs["W1"], inputs["att_src1"], inputs["att_dst1"],
        inputs["b1"], inputs["W2"], inputs["att_src2"], inputs["att_dst2"],
        inputs["b2"], plan)

    if trace:
        _install_ntff_hook()
    res = run_bass_kernel_spmd(nc, in_maps, core_ids=list(range(N_CORES)),
                               trace=trace)
    # undo the slot permutation
    out = np.zeros((N_NODES, C2), np.float32)
    order = plan.order
    for c in range(N_CORES):
        o = res.results[c]["out"]
        ranks = np.arange(SLOTS) * N_CORES + c
        valid = ranks < N_NODES
        out[order[ranks[valid]]] = o[valid]
    return out, res.exec_time_ns


def kernel(**inputs):
    out, _ = kernel_run(inputs)
    return out


# revision 15
# speedup vs baseline: 2.7717x; 1.0073x over previous
"""GAT 2-layer GNN kernel for 8 Trainium2 NeuronCores (v2).

Strategy (graph/data parallel, per the sharding hint):
  - The 50000 dst nodes are dealt round-robin from a global in-degree sort
    into 8 cores x 6272 slots (49 blocks of 128), so every block holds
    near-equal degrees on all cores and ELL padding is ~3%.
  - Table rows (per node: [h bf16 | a_src bf16 | pad] in a 256B row) live in
    a chunk-major DRAM table; both layers share one slot assignment, so one
    int16 index stream serves both edge phases.  Indices are SIGNED offsets
    from a mid-table base row, so a single dma_gather run per block covers
    all 50192 rows (no lo/hi split).  Padding slots point at a row filled
    with -60000: exp maps it to 0, so no mask streams are needed.
  - Per super-group (blocks packed to <=96 chunks) one SWDGE gather call
    fetches all edge rows; calls round-robin over 4 SWDGE queues, which
    parallelizes Q7 descriptor generation (measured ~2.9 ns/edge vs ~8
    single-queue).  exp(leakyrelu(a_src+a_dst)) is computed group-wide and
    written back over the a_src bytes, making each chunk's matmul rhs
    [h*alpha | alpha] contiguous; identity-matmul PSUM accumulation then
    yields per-dst [numerator | denominator].
  - Layer-1 results feed h2 = relu(out1) @ W2 per block; the layer-2 table
    is AllGathered in 4 chunk-major slices that overlap the remaining
    layer-1 compute (same for the layer-1 table under phase A).
  - Layer 2 (1 head x 40) repeats the pipeline and fuses log_softmax; the
    host undoes the slot permutation.
"""

import os
import sys

sys.path.insert(0, "/opt/trn_rl_repo")

import numpy as np
import ml_dtypes

import concourse.bacc as bacc
import concourse.mybir as mybir
from concourse import tile
from concourse.bass_utils import run_bass_kernel_spmd
from concourse.masks import make_identity

bf16 = ml_dtypes.bfloat16

N_NODES = 50000
F_IN = 512
H1 = 8
HID = 8
D1 = H1 * HID  # 64
C2 = 40
N_CORES = 8
BLK = 128
NB = 49
SLOTS = NB * BLK  # 6272
REAL_ROWS = N_CORES * SLOTS  # 50176
PAD_ROWS = 16
T_ROWS = REAL_ROWS + PAD_ROWS  # 50192
IDX_BASE = T_ROWS - 32768  # 17424
PAD_IDX = REAL_ROWS - IDX_BASE  # 32752 (the -60000 row)
TROW = 128  # table row: 128 bf16 = 256 bytes
NEG_SLOPE = 0.2
NEG_FILL = -60000.0
GMAX = 88  # max chunks per gather super-group
BPC = [13, 13, 13, 10]  # AllGather blocks per chunk
AG_LAG = 6  # blocks of emission lag before an AG chunk (head-of-line)
NQ = 4  # SWDGE queues

f32 = mybir.dt.float32
bfl = mybir.dt.bfloat16
i16 = mybir.dt.int16

_CACHE = {}


def _install_ntff_hook():
    """Provide antenv.axon_hooks if the image lacks it, driving NTFF
    profiling via the injected libaxon_pjrt.so C ABI (see trn_boot)."""
    try:
        from antenv.axon_hooks import get_axon_ntff_profile_hook  # noqa: F401
        return
    except ImportError:
        pass
    import contextlib
    import ctypes
    import types

    so_path = "/opt/axon/libaxon_pjrt.so"
    try:
        lib = ctypes.CDLL(so_path)
    except OSError:
        return
    if not hasattr(lib, "axon_start_nrt_profile"):
        return
    lib.axon_start_nrt_profile.argtypes = [ctypes.POINTER(ctypes.c_int64),
                                           ctypes.c_size_t]
    lib.axon_start_nrt_profile.restype = ctypes.c_int64
    lib.axon_stop_nrt_profile.argtypes = [ctypes.c_char_p]
    lib.axon_stop_nrt_profile.restype = ctypes.c_int64

    @contextlib.contextmanager
    def _hook(output_dir, device_ids):
        import jax
        jax.devices()
        if device_ids:
            ids = (ctypes.c_int64 * len(device_ids))(*device_ids)
            rc = lib.axon_start_nrt_profile(ids, len(device_ids))
        else:
            rc = lib.axon_start_nrt_profile(None, 0)
        if rc != 0:
            raise RuntimeError(f"axon_start_nrt_profile rc={rc}")
        try:
            yield
        finally:
            n = lib.axon_stop_nrt_profile(str(output_dir).encode())
            print(f"ntff profile: {n} file(s) written to {output_dir}")

    import antenv
    mod = types.ModuleType("antenv.axon_hooks")
    mod.get_axon_ntff_profile_hook = lambda: _hook
    mod.set_axon_ntff_profile_hook = lambda h: None
    sys.modules["antenv.axon_hooks"] = mod
    antenv.axon_hooks = mod


class Plan:
    pass


def _chunk_major_rows():
    """row(core, slot) for the chunk-major table layout."""
    rows_k = np.array(BPC) * BLK
    chunk_base = np.concatenate([[0], np.cumsum(N_CORES * rows_k)[:-1]])
    sb0 = np.concatenate([[0], np.cumsum(BPC)[:-1]])
    cum_b = np.cumsum(BPC)
    return rows_k, chunk_base, sb0, cum_b


def _prep(edge_index):
    src = np.asarray(edge_index[0], dtype=np.int64)
    dst = np.asarray(edge_index[1], dtype=np.int64)
    loops = np.arange(N_NODES, dtype=np.int64)
    src = np.concatenate([src, loops])
    dst = np.concatenate([dst, loops])

    plan = Plan()
    ktot = np.bincount(dst, minlength=N_NODES)
    order = np.argsort(-ktot, kind="stable")
    core_of = np.empty(N_NODES, np.int64)
    slot_of = np.empty(N_NODES, np.int64)
    core_of[order] = np.arange(N_NODES) % N_CORES
    slot_of[order] = np.arange(N_NODES) // N_CORES

    rows_k, chunk_base, sb0, cum_b = _chunk_major_rows()
    blk_of_slot = slot_of // BLK
    k_of = np.searchsorted(cum_b, blk_of_slot, side="right")
    row_of = (chunk_base[k_of] + core_of * rows_k[k_of]
              + (slot_of - sb0[k_of] * BLK))

    # per-(core,slot) degree -> per-block chunk count
    kP = np.zeros((N_CORES, SLOTS), np.int64)
    kP[core_of, slot_of] = ktot
    nch = kP.reshape(N_CORES, NB, BLK).max(axis=(0, 2))

    # super-groups: pack consecutive blocks, <= GMAX chunks; +1 guard chunk
    groups = []  # (blocks, goff, gch_total, {b: local chunk offset})
    chunk_start = np.zeros(NB, np.int64)  # slab chunk index of block's run
    goff = 0
    b = 0
    while b < NB:
        blocks = [b]
        tot = int(nch[b])
        b += 1
        while b < NB and tot + int(nch[b]) <= GMAX:
            blocks.append(b)
            tot += int(nch[b])
            b += 1
        ch = goff
        for blk in blocks:
            chunk_start[blk] = ch
            ch += int(nch[blk])
        groups.append((blocks, goff, tot + 1))  # +1 guard chunk
        goff += tot + 1
    tch = goff

    # per-core index slabs
    idx_slabs = []
    for c in range(N_CORES):
        sel = core_of[dst] == c
        e_src = src[sel]
        e_slot = slot_of[dst[sel]]
        o = np.argsort(e_slot, kind="stable")
        e_src = e_src[o]
        e_slot = e_slot[o]
        pos = _running_count(e_slot)
        p = e_slot % BLK
        blk = e_slot // BLK
        slabpos = (chunk_start[blk] + pos) * BLK + p
        slab = np.full(tch * BLK, PAD_IDX, np.int16)
        slab[slabpos] = (row_of[e_src] - IDX_BASE).astype(np.int16)
        slab_w = np.tile(slab.reshape(tch * 8, 16).T, (8, 1)).copy()
        idx_slabs.append(slab_w)

    plan.nch = nch
    plan.groups = groups
    plan.tch = tch
    plan.chunk_start = chunk_start
    plan.idx_slabs = idx_slabs
    plan.core_of = core_of
    plan.slot_of = slot_of
    plan.order = order
    return plan


def _running_count(k):
    """pos[i] = number of j<i with k[j]==k[i]; k is sorted."""
    n = len(k)
    if n == 0:
        return np.zeros(0, np.int64)
    starts = np.r_[0, np.flatnonzero(np.diff(k)) + 1]
    run_id = np.zeros(n, np.int64)
    run_id[starts[1:]] = 1
    run_id = np.cumsum(run_id)
    return np.arange(n) - starts[run_id]


def _build(plan):
    nc = bacc.Bacc("TRN2", target_bir_lowering=False, debug=False,
                   num_devices=N_CORES, num_swdge_queues=NQ)

    xT_ext = nc.declare_dram_parameter("xT", [F_IN, SLOTS], bfl, isOutput=False)
    w1_ext = nc.declare_dram_parameter("w1r", [128, 4 * D1], bfl, isOutput=False)
    w2_ext = nc.declare_dram_parameter("w2", [D1, C2], bfl, isOutput=False)
    a1s_ext = nc.declare_dram_parameter("a1srep", [128, D1], f32, isOutput=False)
    a1d_ext = nc.declare_dram_parameter("a1drep", [128, D1], f32, isOutput=False)
    a2s_ext = nc.declare_dram_parameter("a2srep", [128, C2], f32, isOutput=False)
    a2d_ext = nc.declare_dram_parameter("a2drep", [128, C2], f32, isOutput=False)
    b1_ext = nc.declare_dram_parameter("b1rep", [128, D1], f32, isOutput=False)
    b2_ext = nc.declare_dram_parameter("b2rep", [128, C2], f32, isOutput=False)
    idx_ext = nc.declare_dram_parameter("idxs", [128, plan.tch * 8], i16,
                                        isOutput=False)
    out_ext = nc.declare_dram_parameter("out", [SLOTS, C2], f32, isOutput=True)

    t1_shard = nc.dram_tensor("t1_shard", [SLOTS, TROW], bfl)
    t1_full = nc.dram_tensor("t1_full", [T_ROWS, TROW], bfl, addr_space="Shared")
    t2_shard = nc.dram_tensor("t2_shard", [SLOTS, TROW], bfl)
    t2_full = nc.dram_tensor("t2_full", [T_ROWS, TROW], bfl, addr_space="Shared")

    rg = [list(range(N_CORES))]
    rows_k, chunk_base, sb0, _ = _chunk_major_rows()

    def ag_chunks(shard, full):
        """(shard_slice, full_slice) per AllGather chunk."""
        out = []
        for k in range(len(BPC)):
            s0 = sb0[k] * BLK
            s1 = s0 + rows_k[k]
            f0 = chunk_base[k]
            f1 = f0 + N_CORES * rows_k[k]
            out.append((shard[int(s0):int(s1), :], full[int(f0):int(f1), :]))
        return out

    ag1 = ag_chunks(t1_shard, t1_full)
    ag2 = ag_chunks(t2_shard, t2_full)
    # block index after which AG chunk k's rows are complete
    ag_after = np.cumsum(BPC) - 1  # [12, 25, 38, 48]

    with tile.TileContext(nc) as tc:
        with tc.tile_pool(name="const", bufs=1) as cpool:
            ident = cpool.tile([128, 128], bfl)
            make_identity(nc, ident[:, :])
            a1s_t = cpool.tile([128, D1], f32)
            nc.sync.dma_start(out=a1s_t[:, :], in_=a1s_ext[:, :])
            a1d_t = cpool.tile([128, D1], f32)
            nc.sync.dma_start(out=a1d_t[:, :], in_=a1d_ext[:, :])
            a2s_t = cpool.tile([128, C2], f32)
            nc.sync.dma_start(out=a2s_t[:, :], in_=a2s_ext[:, :])
            a2d_t = cpool.tile([128, C2], f32)
            nc.sync.dma_start(out=a2d_t[:, :], in_=a2d_ext[:, :])
            b1_t = cpool.tile([128, D1], f32)
            nc.sync.dma_start(out=b1_t[:, :], in_=b1_ext[:, :])
            b2_t = cpool.tile([128, C2], f32)
            nc.sync.dma_start(out=b2_t[:, :], in_=b2_ext[:, :])
            w2_t = cpool.tile([D1, C2], bfl)
            nc.sync.dma_start(out=w2_t[:, :], in_=w2_ext[:, :])
            idx_t = cpool.tile([128, plan.tch * 8], i16)
            nc.sync.dma_start(out=idx_t[:, :], in_=idx_ext[:, :])
            adst1 = cpool.tile([128, NB, H1], f32)
            adst2 = cpool.tile([128, NB, 1], f32)

            # -60000 pad rows, written locally into both tables
            neg_t = cpool.tile([PAD_ROWS, TROW], bfl)
            nc.vector.memset(neg_t[:, :], NEG_FILL)
            nc.sync.dma_start(out=t1_full[REAL_ROWS:T_ROWS, :], in_=neg_t[:, :])
            nc.sync.dma_start(out=t2_full[REAL_ROWS:T_ROWS, :], in_=neg_t[:, :])

            # ---------------- Phase A: h1 = x @ W1, a_src1/a_dst1 ----------
            with tc.tile_pool(name="phA", bufs=2) as apool, \
                 tc.tile_pool(name="phA_ps", bufs=2, space="PSUM") as apsum:
                w1_t = apool.tile([128, 4, D1], bfl, tag="w1")
                nc.sync.dma_start(out=w1_t[:, :, :], in_=w1_ext[:, :])
                xk = []
                for k in range(4):
                    xt = apool.tile([128, SLOTS], bfl, tag=f"xk{k}")
                    nc.sync.dma_start(out=xt[:, :],
                                      in_=xT_ext[k * 128:(k + 1) * 128, :])
                    xk.append(xt)
                agq1 = list(ag1)
                for b in range(NB):
                    hps = apsum.tile([128, D1], f32, tag="hps")
                    for k in range(4):
                        nc.tensor.matmul(
                            hps[:, :], lhsT=xk[k][:, b * BLK:(b + 1) * BLK],
                            rhs=w1_t[:, k, :], start=(k == 0), stop=(k == 3))
                    t1row = apool.tile([128, 96], bfl, tag="t1row")
                    nc.scalar.activation(out=t1row[:, 0:D1], in_=hps[:, :],
                                         func=mybir.ActivationFunctionType.Copy)
                    tmp = apool.tile([128, D1], f32, tag="atmp")
                    nc.vector.tensor_tensor(out=tmp[:, :], in0=hps[:, :],
                                            in1=a1s_t[:, :],
                                            op=mybir.AluOpType.mult)
                    nc.vector.tensor_reduce(
                        out=t1row[:, D1:D1 + 2 * H1].bitcast(f32),
                        in_=tmp[:, :].rearrange("p (h c) -> p h c", h=H1, c=HID),
                        axis=mybir.AxisListType.X, op=mybir.AluOpType.add)
                    tmp2 = apool.tile([128, D1], f32, tag="atmp2")
                    nc.vector.tensor_tensor(out=tmp2[:, :], in0=hps[:, :],
                                            in1=a1d_t[:, :],
                                            op=mybir.AluOpType.mult)
                    nc.vector.tensor_reduce(
                        out=adst1[:, b, :],
                        in_=tmp2[:, :].rearrange("p (h c) -> p h c", h=H1, c=HID),
                        axis=mybir.AxisListType.X, op=mybir.AluOpType.add)
                    nc.sync.dma_start(
                        out=t1_shard[b * BLK:(b + 1) * BLK, 0:96],
                        in_=t1row[:, :])
                    if agq1 and b >= ag_after[len(ag1) - len(agq1)] + AG_LAG:
                        ins_ap, outs_ap = agq1.pop(0)
                        nc.gpsimd.collective_compute(
                            "AllGather", mybir.AluOpType.bypass,
                            replica_groups=rg,
                            ins=[ins_ap.opt()], outs=[outs_ap.opt()])
                while agq1:
                    ins_ap, outs_ap = agq1.pop(0)
                    nc.gpsimd.collective_compute(
                        "AllGather", mybir.AluOpType.bypass, replica_groups=rg,
                        ins=[ins_ap.opt()], outs=[outs_ap.opt()])

            # gathers read below their declared in_ap slice (signed idxs), so
            # the AG->gather dependency must be a hard barrier
            tc.strict_bb_all_engine_barrier()

            _edge_phase(nc, tc, plan, layer=1, table_full=t1_full, idx_t=idx_t,
                        ident=ident, adst=adst1, bias=b1_t, w2_t=w2_t,
                        a2s_t=a2s_t, a2d_t=a2d_t, adst_next=adst2,
                        t_next_shard=t2_shard, ag_next=ag2, ag_after=ag_after,
                        rg=rg, b2_t=None, out_ext=None)

            tc.strict_bb_all_engine_barrier()

            _edge_phase(nc, tc, plan, layer=2, table_full=t2_full, idx_t=idx_t,
                        ident=ident, adst=adst2, bias=None, w2_t=None,
                        a2s_t=None, a2d_t=None, adst_next=None,
                        t_next_shard=None, ag_next=None, ag_after=None,
                        rg=None, b2_t=b2_t, out_ext=out_ext)

    nc.compile()
    return nc


def _edge_phase(nc, tc, plan, layer, table_full, idx_t, ident, adst, bias,
                w2_t, a2s_t, a2d_t, adst_next, t_next_shard, ag_next, ag_after,
                rg, b2_t, out_ext):
    if layer == 1:
        NH, CH, CC = H1, HID, D1   # 8 heads x 8 ch
    else:
        NH, CH, CC = 1, C2, C2     # 1 head x 40
    NCOL = CC + NH
    base = table_full[IDX_BASE:T_ROWS, :]
    agq = list(ag_next) if ag_next else []
    qrr = [0]
    ACT = mybir.ActivationFunctionType

    with tc.tile_pool(name=f"e{layer}", bufs=4) as pool, \
         tc.tile_pool(name=f"e{layer}t", bufs=3) as tpool, \
         tc.tile_pool(name=f"e{layer}s", bufs=2) as spool, \
         tc.tile_pool(name=f"e{layer}_ps", bufs=2, space="PSUM") as psum, \
         tc.tile_pool(name=f"e{layer}_ps2", bufs=2, space="PSUM") as psum2:
        for blocks, goff, gch in plan.groups:
            nblk = len(blocks)
            g_t = pool.tile([128, gch, TROW], bfl, tag="gath")
            nc.gpsimd.dma_gather(
                out_ap=g_t[:, :, :], in_ap=base,
                idxs_ap=idx_t[:, goff * 8:(goff + gch) * 8],
                num_idxs=gch * BLK, num_idxs_reg=gch * BLK,
                elem_size=TROW, single_packet=False,
                queue_num=qrr[0] % NQ)
            qrr[0] += 1
            nreal = gch - 1  # last chunk is the guard

            # group-wide alpha: exp(leakyrelu(a_src + a_dst)) -> a_src slot
            e_t = tpool.tile([128, nreal, NH], f32, tag="elog")
            for blk in blocks:
                r0 = int(plan.chunk_start[blk]) - goff
                nchb = int(plan.nch[blk])
                nc.vector.tensor_tensor(
                    out=e_t[:, r0:r0 + nchb, :],
                    in0=g_t[:, r0:r0 + nchb, CC:CC + 2 * NH].bitcast(f32),
                    in1=adst[:, blk, None, :].to_broadcast([128, nchb, NH]),
                    op=mybir.AluOpType.add)
            lr_t = tpool.tile([128, nreal, NH], f32, tag="lrelu")
            nc.vector.scalar_tensor_tensor(
                out=lr_t[:, :, :], in0=e_t[:, :, :], scalar=NEG_SLOPE,
                in1=e_t[:, :, :], op0=mybir.AluOpType.mult,
                op1=mybir.AluOpType.max)
            nc.scalar.activation(out=g_t[:, 0:nreal, CC:CC + NH],
                                 in_=lr_t[:, :, :], func=ACT.Exp)
            nc.vector.tensor_tensor(
                out=g_t[:, 0:nreal, 0:CC].rearrange(
                    "p g (h c) -> p g h c", h=NH, c=CH),
                in0=g_t[:, 0:nreal, 0:CC].rearrange(
                    "p g (h c) -> p g h c", h=NH, c=CH),
                in1=g_t[:, 0:nreal, CC:CC + NH, None].to_broadcast(
                    [128, nreal, NH, CH]),
                op=mybir.AluOpType.mult)

            for blk in blocks:
                r0 = int(plan.chunk_start[blk]) - goff
                nchb = int(plan.nch[blk])
                ps = psum.tile([128, NCOL], f32, tag="agg")
                for j in range(nchb):
                    nc.tensor.matmul(ps[:, :], lhsT=ident[:, :],
                                     rhs=g_t[:, r0 + j, 0:NCOL],
                                     start=(j == 0), stop=(j == nchb - 1))

                pg = spool.tile([128, NCOL], f32, tag="pg")
                nc.scalar.activation(out=pg[:, :], in_=ps[:, :], func=ACT.Copy)
                recip = spool.tile([128, NH], f32, tag="recip")
                nc.vector.reciprocal(out=recip[:, :], in_=pg[:, CC:NCOL])
                o_t = spool.tile([128, CC], f32, tag="outb")
                if layer == 1:
                    nc.vector.tensor_tensor(
                        out=o_t[:, :].rearrange("p (h c) -> p h c",
                                                h=NH, c=CH),
                        in0=pg[:, 0:CC].rearrange("p (h c) -> p h c",
                                                  h=NH, c=CH),
                        in1=recip[:, :, None].to_broadcast([128, NH, CH]),
                        op=mybir.AluOpType.mult)

                if layer == 1:
                    obt = spool.tile([128, CC], f32, tag="outbt")
                    nc.vector.tensor_tensor(out=obt[:, :], in0=o_t[:, :],
                                            in1=bias[:, :],
                                            op=mybir.AluOpType.add)
                    ob = spool.tile([128, CC], bfl, tag="outbf")
                    nc.vector.tensor_scalar(out=ob[:, :], in0=obt[:, :],
                                            scalar1=0.0, scalar2=None,
                                            op0=mybir.AluOpType.max)
                    tps = psum2.tile([D1, 128], bfl, tag="tp")
                    nc.tensor.transpose(tps[:, :], ob[:, :], ident[:, :])
                    h1T = spool.tile([D1, 128], bfl, tag="h1T")
                    nc.vector.tensor_copy(out=h1T[:, :], in_=tps[:, :])
                    h2ps = psum2.tile([128, C2], f32, tag="h2")
                    nc.tensor.matmul(h2ps[:, :], lhsT=h1T[:, :], rhs=w2_t[:, :],
                                     start=True, stop=True)
                    t2row = spool.tile([128, 48], bfl, tag="t2row")
                    nc.scalar.activation(out=t2row[:, 0:C2], in_=h2ps[:, :],
                                         func=ACT.Copy)
                    t1 = spool.tile([128, C2], f32, tag="t1")
                    nc.vector.tensor_tensor(out=t1[:, :], in0=h2ps[:, :],
                                            in1=a2s_t[:, :],
                                            op=mybir.AluOpType.mult)
                    nc.vector.tensor_reduce(
                        out=t2row[:, C2:C2 + 2].bitcast(f32), in_=t1[:, :],
                        axis=mybir.AxisListType.X, op=mybir.AluOpType.add)
                    t2 = spool.tile([128, C2], f32, tag="t2")
                    nc.vector.tensor_tensor(out=t2[:, :], in0=h2ps[:, :],
                                            in1=a2d_t[:, :],
                                            op=mybir.AluOpType.mult)
                    nc.vector.tensor_reduce(
                        out=adst_next[:, blk, :], in_=t2[:, :],
                        axis=mybir.AxisListType.X, op=mybir.AluOpType.add)
                    nc.sync.dma_start(
                        out=t_next_shard[blk * BLK:(blk + 1) * BLK, 0:48],
                        in_=t2row[:, :])
                else:
                    lg = spool.tile([128, C2], f32, tag="logits")
                    nc.vector.scalar_tensor_tensor(
                        out=lg[:, :], in0=pg[:, 0:CC], scalar=recip[:, 0:1],
                        in1=b2_t[:, :], op0=mybir.AluOpType.mult,
                        op1=mybir.AluOpType.add)
                    negm = spool.tile([128, 1], f32, tag="negm")
                    nc.vector.tensor_reduce(out=negm[:, :], in_=lg[:, :],
                                            axis=mybir.AxisListType.X,
                                            op=mybir.AluOpType.max, negate=True)
                    ex = spool.tile([128, C2], f32, tag="sfex")
                    ssum = spool.tile([128, 1], f32, tag="ssum")
                    nc.scalar.activation(out=ex[:, :], in_=lg[:, :],
                                         func=ACT.Exp,
                                         bias=negm[:, :], accum_out=ssum[:, :])
                    lse = spool.tile([128, 1], f32, tag="lse")
                    nc.scalar.activation(out=lse[:, :], in_=ssum[:, :],
                                         func=ACT.Ln)
                    res = spool.tile([128, C2], f32, tag="res")
                    nc.vector.scalar_tensor_tensor(
                        out=res[:, :], in0=lg[:, :], scalar=negm[:, :],
                        in1=lse[:, :].to_broadcast([128, C2]),
                        op0=mybir.AluOpType.add, op1=mybir.AluOpType.subtract)
                    nc.sync.dma_start(out=out_ext[blk * BLK:(blk + 1) * BLK, :],
                                      in_=res[:, :])

            last_blk = blocks[-1]
            while agq and last_blk >= ag_after[len(ag_next) - len(agq)] + AG_LAG:
                ins_ap, outs_ap = agq.pop(0)
                nc.gpsimd.collective_compute(
                    "AllGather", mybir.AluOpType.bypass, replica_groups=rg,
                    ins=[ins_ap.opt()], outs=[outs_ap.opt()])
        while agq:
            ins_ap, outs_ap = agq.pop(0)
            nc.gpsimd.collective_compute(
                "AllGather", mybir.AluOpType.bypass, replica_groups=rg,
                ins=[ins_ap.opt()], outs=[outs_ap.opt()])


def _host_inputs(x, W1, att_src1, att_dst1, b1, W2, att_src2, att_dst2, b2,
                 plan):
    w1r = np.ascontiguousarray(
        np.asarray(W1, np.float32).reshape(4, 128, D1).transpose(1, 0, 2)
    ).reshape(128, 4 * D1).astype(bf16)
    rep = lambda v, n: np.tile(np.asarray(v, np.float32).reshape(1, n),
                               (128, 1)).astype(np.float32)
    x32 = np.asarray(x, np.float32)

    # xT per core: column s = x[node at (c, s)].T
    order = plan.order
    in_maps = []
    for c in range(N_CORES):
        ranks = np.arange(SLOTS) * N_CORES + c
        valid = ranks < N_NODES
        nodes = order[ranks[valid]]
        xT = np.zeros((F_IN, SLOTS), bf16)
        xT[:, valid] = x32[nodes].T.astype(bf16)
        in_maps.append({
            "xT": xT,
            "w1r": w1r,
            "w2": np.asarray(W2, np.float32).astype(bf16),
            "a1srep": rep(att_src1, D1),
            "a1drep": rep(att_dst1, D1),
            "a2srep": rep(att_src2, C2),
            "a2drep": rep(att_dst2, C2),
            "b1rep": rep(b1, D1),
            "b2rep": rep(b2, C2),
            "idxs": plan.idx_slabs[c],
        })
    return in_maps


def kernel_run(inputs, trace=False):
    """Build (cached), run, and return (out [50000,40] f32, exec_time_ns)."""
    edge_index = inputs["edge_index"]
    plan = _prep(edge_index)

    key = tuple(plan.nch)
    if key not in _CACHE:
        _CACHE[key] = _build(plan)
    nc = _CACHE[key]

    in_maps = _host_inputs(
        inputs["x"], inputs["W1"], inputs["att_src1"], inputs["att_dst1"],
        inputs["b1"], inputs["W2"], inputs["att_src2"], inputs["att_dst2"],
        inputs["b2"], plan)

    if trace:
        _install_ntff_hook()
    res = run_bass_kernel_spmd(nc, in_maps, core_ids=list(range(N_CORES)),
                               trace=trace)
    # undo the slot permutation
    out = np.zeros((N_NODES, C2), np.float32)
    order = plan.order
    for c in range(N_CORES):
        o = res.results[c]["out"]
        ranks = np.arange(SLOTS) * N_CORES + c
        valid = ranks < N_NODES
        out[order[ranks[valid]]] = o[valid]
    return out, res.exec_time_ns


def kernel(**inputs):
    out, _ = kernel_run(inputs)
    return out


# revision 22
# speedup vs baseline: 3.2695x; 1.1796x over previous
"""GAT 2-layer GNN kernel for 8 Trainium2 NeuronCores (v2).

Strategy (graph/data parallel, per the sharding hint):
  - The 50000 dst nodes are dealt round-robin from a global in-degree sort
    into 8 cores x 6272 slots (49 blocks of 128), so every block holds
    near-equal degrees on all cores and ELL padding is ~3%.
  - Table rows (per node: [h bf16 | a_src bf16 | pad] in a 256B row) live in
    a chunk-major DRAM table; both layers share one slot assignment, so one
    int16 index stream serves both edge phases.  Indices are SIGNED offsets
    from a mid-table base row, so a single dma_gather run per block covers
    all 50192 rows (no lo/hi split).  Padding slots point at a row filled
    with -60000: exp maps it to 0, so no mask streams are needed.
  - Per super-group (blocks packed to <=96 chunks) one SWDGE gather call
    fetches all edge rows; calls round-robin over 4 SWDGE queues, which
    parallelizes Q7 descriptor generation (measured ~2.9 ns/edge vs ~8
    single-queue).  exp(leakyrelu(a_src+a_dst)) is computed group-wide and
    written back over the a_src bytes, making each chunk's matmul rhs
    [h*alpha | alpha] contiguous; identity-matmul PSUM accumulation then
    yields per-dst [numerator | denominator].
  - Layer-1 results feed h2 = relu(out1) @ W2 per block; the layer-2 table
    is AllGathered in 4 chunk-major slices that overlap the remaining
    layer-1 compute (same for the layer-1 table under phase A).
  - Layer 2 (1 head x 40) repeats the pipeline and fuses log_softmax; the
    host undoes the slot permutation.
"""

import os
import sys

sys.path.insert(0, "/opt/trn_rl_repo")

import numpy as np
import ml_dtypes

import concourse.bacc as bacc
import concourse.mybir as mybir
from concourse import tile
from concourse.bass_utils import run_bass_kernel_spmd
from concourse.masks import make_identity

bf16 = ml_dtypes.bfloat16

N_NODES = 50000
F_IN = 512
H1 = 8
HID = 8
D1 = H1 * HID  # 64
C2 = 40
N_CORES = 8
BLK = 128
NB = 49
SLOTS = NB * BLK  # 6272
REAL_ROWS = N_CORES * SLOTS  # 50176
PAD_ROWS = 16
T_ROWS = REAL_ROWS + PAD_ROWS  # 50192
IDX_BASE = T_ROWS - 32768  # 17424
PAD_IDX = REAL_ROWS - IDX_BASE  # 32752 (the -60000 row)
TROW = 128  # table row: 128 bf16 = 256 bytes
NEG_SLOPE = 0.2
NEG_FILL = -50.0
GMAX = 88  # max chunks per gather super-group
BPC = [15, 15, 15, 4]  # AllGather blocks per chunk
AG_LAG = 6  # blocks of emission lag before an AG chunk (head-of-line)
NQ = 4  # SWDGE queues

f32 = mybir.dt.float32
bfl = mybir.dt.bfloat16
i16 = mybir.dt.int16

_CACHE = {}


def _install_ntff_hook():
    """Provide antenv.axon_hooks if the image lacks it, driving NTFF
    profiling via the injected libaxon_pjrt.so C ABI (see trn_boot)."""
    try:
        from antenv.axon_hooks import get_axon_ntff_profile_hook  # noqa: F401
        return
    except ImportError:
        pass
    import contextlib
    import ctypes
    import types

    so_path = "/opt/axon/libaxon_pjrt.so"
    try:
        lib = ctypes.CDLL(so_path)
    except OSError:
        return
    if not hasattr(lib, "axon_start_nrt_profile"):
        return
    lib.axon_start_nrt_profile.argtypes = [ctypes.POINTER(ctypes.c_int64),
                                           ctypes.c_size_t]
    lib.axon_start_nrt_profile.restype = ctypes.c_int64
    lib.axon_stop_nrt_profile.argtypes = [ctypes.c_char_p]
    lib.axon_stop_nrt_profile.restype = ctypes.c_int64

    @contextlib.contextmanager
    def _hook(output_dir, device_ids):
        import jax
        jax.devices()
        if device_ids:
            ids = (ctypes.c_int64 * len(device_ids))(*device_ids)
            rc = lib.axon_start_nrt_profile(ids, len(device_ids))
        else:
            rc = lib.axon_start_nrt_profile(None, 0)
        if rc != 0:
            raise RuntimeError(f"axon_start_nrt_profile rc={rc}")
        try:
            yield
        finally:
            n = lib.axon_stop_nrt_profile(str(output_dir).encode())
            print(f"ntff profile: {n} file(s) written to {output_dir}")

    import antenv
    mod = types.ModuleType("antenv.axon_hooks")
    mod.get_axon_ntff_profile_hook = lambda: _hook
    mod.set_axon_ntff_profile_hook = lambda h: None
    sys.modules["antenv.axon_hooks"] = mod
    antenv.axon_hooks = mod


class Plan:
    pass


def _chunk_major_rows():
    """row(core, slot) for the chunk-major table layout."""
    rows_k = np.array(BPC) * BLK
    chunk_base = np.concatenate([[0], np.cumsum(N_CORES * rows_k)[:-1]])
    sb0 = np.concatenate([[0], np.cumsum(BPC)[:-1]])
    cum_b = np.cumsum(BPC)
    return rows_k, chunk_base, sb0, cum_b


def _prep(edge_index):
    src = np.asarray(edge_index[0], dtype=np.int64)
    dst = np.asarray(edge_index[1], dtype=np.int64)
    loops = np.arange(N_NODES, dtype=np.int64)
    src = np.concatenate([src, loops])
    dst = np.concatenate([dst, loops])

    plan = Plan()
    ktot = np.bincount(dst, minlength=N_NODES)
    order = np.argsort(-ktot, kind="stable")
    core_of = np.empty(N_NODES, np.int64)
    slot_of = np.empty(N_NODES, np.int64)
    core_of[order] = np.arange(N_NODES) % N_CORES
    slot_of[order] = np.arange(N_NODES) // N_CORES

    rows_k, chunk_base, sb0, cum_b = _chunk_major_rows()
    blk_of_slot = slot_of // BLK
    k_of = np.searchsorted(cum_b, blk_of_slot, side="right")
    row_of = (chunk_base[k_of] + core_of * rows_k[k_of]
              + (slot_of - sb0[k_of] * BLK))

    # per-(core,slot) degree -> per-block chunk count
    kP = np.zeros((N_CORES, SLOTS), np.int64)
    kP[core_of, slot_of] = ktot
    nch = kP.reshape(N_CORES, NB, BLK).max(axis=(0, 2))

    # super-groups: pack consecutive blocks, <= GMAX chunks; +1 guard chunk
    groups = []  # (blocks, goff, gch_total, {b: local chunk offset})
    chunk_start = np.zeros(NB, np.int64)  # slab chunk index of block's run
    goff = 0
    b = 0
    while b < NB:
        blocks = [b]
        tot = int(nch[b])
        b += 1
        while b < NB and tot + int(nch[b]) <= GMAX:
            blocks.append(b)
            tot += int(nch[b])
            b += 1
        ch = goff
        for blk in blocks:
            chunk_start[blk] = ch
            ch += int(nch[blk])
        groups.append((blocks, goff, tot + 1))  # +1 guard chunk
        goff += tot + 1
    tch = goff

    # per-core index slabs
    idx_slabs = []
    for c in range(N_CORES):
        sel = core_of[dst] == c
        e_src = src[sel]
        e_slot = slot_of[dst[sel]]
        o = np.argsort(e_slot, kind="stable")
        e_src = e_src[o]
        e_slot = e_slot[o]
        pos = _running_count(e_slot)
        p = e_slot % BLK
        blk = e_slot // BLK
        slabpos = (chunk_start[blk] + pos) * BLK + p
        slab = np.full(tch * BLK, PAD_IDX, np.int16)
        slab[slabpos] = (row_of[e_src] - IDX_BASE).astype(np.int16)
        slab_w = np.tile(slab.reshape(tch * 8, 16).T, (8, 1)).copy()
        idx_slabs.append(slab_w)

    plan.nch = nch
    plan.groups = groups
    plan.tch = tch
    plan.chunk_start = chunk_start
    plan.idx_slabs = idx_slabs
    plan.core_of = core_of
    plan.slot_of = slot_of
    plan.order = order
    return plan


def _running_count(k):
    """pos[i] = number of j<i with k[j]==k[i]; k is sorted."""
    n = len(k)
    if n == 0:
        return np.zeros(0, np.int64)
    starts = np.r_[0, np.flatnonzero(np.diff(k)) + 1]
    run_id = np.zeros(n, np.int64)
    run_id[starts[1:]] = 1
    run_id = np.cumsum(run_id)
    return np.arange(n) - starts[run_id]


def _build(plan):
    nc = bacc.Bacc("TRN2", target_bir_lowering=False, debug=False,
                   num_devices=N_CORES, num_swdge_queues=NQ)

    xT_ext = nc.declare_dram_parameter("xT", [F_IN, SLOTS], bfl, isOutput=False)
    w1_ext = nc.declare_dram_parameter("w1r", [128, 4 * D1], bfl, isOutput=False)
    w2_ext = nc.declare_dram_parameter("w2", [D1, C2], bfl, isOutput=False)
    a1s_ext = nc.declare_dram_parameter("a1srep", [128, D1], f32, isOutput=False)
    a1d_ext = nc.declare_dram_parameter("a1drep", [128, D1], f32, isOutput=False)
    a2s_ext = nc.declare_dram_parameter("a2srep", [128, C2], f32, isOutput=False)
    a2d_ext = nc.declare_dram_parameter("a2drep", [128, C2], f32, isOutput=False)
    b1_ext = nc.declare_dram_parameter("b1rep", [128, D1], f32, isOutput=False)
    b2_ext = nc.declare_dram_parameter("b2rep", [128, C2], f32, isOutput=False)
    idx_ext = nc.declare_dram_parameter("idxs", [128, plan.tch * 8], i16,
                                        isOutput=False)
    out_ext = nc.declare_dram_parameter("out", [SLOTS, C2], f32, isOutput=True)

    t1_shard = nc.dram_tensor("t1_shard", [SLOTS, TROW], bfl)
    t1_full = nc.dram_tensor("t1_full", [T_ROWS, TROW], bfl, addr_space="Shared")
    t2_shard = nc.dram_tensor("t2_shard", [SLOTS, TROW], bfl)
    t2_full = nc.dram_tensor("t2_full", [T_ROWS, TROW], bfl, addr_space="Shared")

    rg = [list(range(N_CORES))]
    rows_k, chunk_base, sb0, _ = _chunk_major_rows()

    def ag_chunks(shard, full):
        """(shard_slice, full_slice) per AllGather chunk."""
        out = []
        for k in range(len(BPC)):
            s0 = sb0[k] * BLK
            s1 = s0 + rows_k[k]
            f0 = chunk_base[k]
            f1 = f0 + N_CORES * rows_k[k]
            out.append((shard[int(s0):int(s1), :], full[int(f0):int(f1), :]))
        return out

    ag1 = ag_chunks(t1_shard, t1_full)
    ag2 = ag_chunks(t2_shard, t2_full)
    # block index after which AG chunk k's rows are complete
    ag_after = np.cumsum(BPC) - 1  # [12, 25, 38, 48]

    with tile.TileContext(nc) as tc:
        with tc.tile_pool(name="const", bufs=1) as cpool:
            ident = cpool.tile([128, 128], bfl)
            make_identity(nc, ident[:, :])
            a1s_t = cpool.tile([128, D1], f32)
            nc.sync.dma_start(out=a1s_t[:, :], in_=a1s_ext[:, :])
            a1d_t = cpool.tile([128, D1], f32)
            nc.sync.dma_start(out=a1d_t[:, :], in_=a1d_ext[:, :])
            a2s_t = cpool.tile([128, C2], f32)
            nc.sync.dma_start(out=a2s_t[:, :], in_=a2s_ext[:, :])
            a2d_t = cpool.tile([128, C2], f32)
            nc.sync.dma_start(out=a2d_t[:, :], in_=a2d_ext[:, :])
            b1_t = cpool.tile([128, D1], f32)
            nc.sync.dma_start(out=b1_t[:, :], in_=b1_ext[:, :])
            b2_t = cpool.tile([128, C2], f32)
            nc.sync.dma_start(out=b2_t[:, :], in_=b2_ext[:, :])
            w2_t = cpool.tile([D1, C2], bfl)
            nc.sync.dma_start(out=w2_t[:, :], in_=w2_ext[:, :])
            idx_t = cpool.tile([128, plan.tch * 8], i16)
            nc.sync.dma_start(out=idx_t[:, :], in_=idx_ext[:, :])
            adst1 = cpool.tile([128, NB, H1], f32)
            adst2 = cpool.tile([128, NB, 1], f32)

            # pad rows: h = 0, a_src = -50 -> alpha ~ 5e-5 (tiny but finite,
            # so padding dsts get finite denominators and no NaN downstream)
            neg1_t = cpool.tile([PAD_ROWS, TROW], bfl)
            nc.vector.memset(neg1_t[:, :], 0.0)
            nc.vector.memset(neg1_t[:, D1:D1 + 2 * H1].bitcast(f32), NEG_FILL)
            nc.sync.dma_start(out=t1_full[REAL_ROWS:T_ROWS, :], in_=neg1_t[:, :])
            neg2_t = cpool.tile([PAD_ROWS, TROW], bfl)
            nc.vector.memset(neg2_t[:, :], 0.0)
            nc.vector.memset(neg2_t[:, C2:C2 + 2].bitcast(f32), NEG_FILL)
            nc.sync.dma_start(out=t2_full[REAL_ROWS:T_ROWS, :], in_=neg2_t[:, :])

            # ---------------- Phase A: h1 = x @ W1, a_src1/a_dst1 ----------
            with tc.tile_pool(name="phA", bufs=2) as apool, \
                 tc.tile_pool(name="phA_ps", bufs=2, space="PSUM") as apsum:
                w1_t = apool.tile([128, 4, D1], bfl, tag="w1")
                nc.sync.dma_start(out=w1_t[:, :, :], in_=w1_ext[:, :])
                xk = []
                for k in range(4):
                    xt = apool.tile([128, SLOTS], bfl, tag=f"xk{k}")
                    nc.sync.dma_start(out=xt[:, :],
                                      in_=xT_ext[k * 128:(k + 1) * 128, :])
                    xk.append(xt)
                agq1 = list(ag1)
                for b in range(NB):
                    hps = apsum.tile([128, D1], f32, tag="hps")
                    for k in range(4):
                        nc.tensor.matmul(
                            hps[:, :], lhsT=xk[k][:, b * BLK:(b + 1) * BLK],
                            rhs=w1_t[:, k, :], start=(k == 0), stop=(k == 3))
                    t1row = apool.tile([128, 96], bfl, tag="t1row")
                    nc.scalar.activation(out=t1row[:, 0:D1], in_=hps[:, :],
                                         func=mybir.ActivationFunctionType.Copy)
                    tmp = apool.tile([128, D1], f32, tag="atmp")
                    nc.vector.tensor_tensor(out=tmp[:, :], in0=hps[:, :],
                                            in1=a1s_t[:, :],
                                            op=mybir.AluOpType.mult)
                    nc.vector.tensor_reduce(
                        out=t1row[:, D1:D1 + 2 * H1].bitcast(f32),
                        in_=tmp[:, :].rearrange("p (h c) -> p h c", h=H1, c=HID),
                        axis=mybir.AxisListType.X, op=mybir.AluOpType.add)
                    tmp2 = apool.tile([128, D1], f32, tag="atmp2")
                    nc.vector.tensor_tensor(out=tmp2[:, :], in0=hps[:, :],
                                            in1=a1d_t[:, :],
                                            op=mybir.AluOpType.mult)
                    nc.vector.tensor_reduce(
                        out=adst1[:, b, :],
                        in_=tmp2[:, :].rearrange("p (h c) -> p h c", h=H1, c=HID),
                        axis=mybir.AxisListType.X, op=mybir.AluOpType.add)
                    nc.sync.dma_start(
                        out=t1_shard[b * BLK:(b + 1) * BLK, 0:96],
                        in_=t1row[:, :])
                    if agq1 and b >= ag_after[len(ag1) - len(agq1)] + AG_LAG:
                        ins_ap, outs_ap = agq1.pop(0)
                        nc.gpsimd.collective_compute(
                            "AllGather", mybir.AluOpType.bypass,
                            replica_groups=rg,
                            ins=[ins_ap.opt()], outs=[outs_ap.opt()])
                while agq1:
                    ins_ap, outs_ap = agq1.pop(0)
                    nc.gpsimd.collective_compute(
                        "AllGather", mybir.AluOpType.bypass, replica_groups=rg,
                        ins=[ins_ap.opt()], outs=[outs_ap.opt()])

            # gathers read below their declared in_ap slice (signed idxs), so
            # the AG->gather dependency must be a hard barrier
            tc.strict_bb_all_engine_barrier()

            _edge_phase(nc, tc, plan, layer=1, table_full=t1_full, idx_t=idx_t,
                        ident=ident, adst=adst1, bias=b1_t, w2_t=w2_t,
                        a2s_t=a2s_t, a2d_t=a2d_t, adst_next=adst2,
                        t_next_shard=t2_shard, ag_next=ag2, ag_after=ag_after,
                        rg=rg, b2_t=None, out_ext=None)

            tc.strict_bb_all_engine_barrier()

            _edge_phase(nc, tc, plan, layer=2, table_full=t2_full, idx_t=idx_t,
                        ident=ident, adst=adst2, bias=None, w2_t=None,
                        a2s_t=None, a2d_t=None, adst_next=None,
                        t_next_shard=None, ag_next=None, ag_after=None,
                        rg=None, b2_t=b2_t, out_ext=out_ext)

    nc.compile()
    return nc


def _edge_phase(nc, tc, plan, layer, table_full, idx_t, ident, adst, bias,
                w2_t, a2s_t, a2d_t, adst_next, t_next_shard, ag_next, ag_after,
                rg, b2_t, out_ext):
    if layer == 1:
        NH, CH, CC = H1, HID, D1   # 8 heads x 8 ch
    else:
        NH, CH, CC = 1, C2, C2     # 1 head x 40
    NCOL = CC + NH
    base = table_full[IDX_BASE:T_ROWS, :]
    agq = list(ag_next) if ag_next else []
    qrr = [0]
    ACT = mybir.ActivationFunctionType

    with tc.tile_pool(name=f"e{layer}", bufs=6) as pool, \
         tc.tile_pool(name=f"e{layer}t", bufs=3) as tpool, \
         tc.tile_pool(name=f"e{layer}s", bufs=2) as spool, \
         tc.tile_pool(name=f"e{layer}_ps", bufs=2, space="PSUM") as psum, \
         tc.tile_pool(name=f"e{layer}_ps2", bufs=2, space="PSUM") as psum2:
        for blocks, goff, gch in plan.groups:
            nblk = len(blocks)
            g_t = pool.tile([128, gch, TROW], bfl, tag="gath")
            nc.gpsimd.dma_gather(
                out_ap=g_t[:, :, :], in_ap=base,
                idxs_ap=idx_t[:, goff * 8:(goff + gch) * 8],
                num_idxs=gch * BLK, num_idxs_reg=gch * BLK,
                elem_size=TROW, single_packet=False,
                queue_num=qrr[0] % NQ)
            qrr[0] += 1
            nreal = gch - 1  # last chunk is the guard

            # group-wide alpha: exp(leakyrelu(a_src + a_dst)) -> a_src slot
            e_t = tpool.tile([128, nreal, NH], f32, tag="elog")
            for blk in blocks:
                r0 = int(plan.chunk_start[blk]) - goff
                nchb = int(plan.nch[blk])
                nc.vector.tensor_tensor(
                    out=e_t[:, r0:r0 + nchb, :],
                    in0=g_t[:, r0:r0 + nchb, CC:CC + 2 * NH].bitcast(f32),
                    in1=adst[:, blk, None, :].to_broadcast([128, nchb, NH]),
                    op=mybir.AluOpType.add)
            lr_t = tpool.tile([128, nreal, NH], f32, tag="lrelu")
            nc.vector.scalar_tensor_tensor(
                out=lr_t[:, :, :], in0=e_t[:, :, :], scalar=NEG_SLOPE,
                in1=e_t[:, :, :], op0=mybir.AluOpType.mult,
                op1=mybir.AluOpType.max)
            nc.scalar.activation(out=g_t[:, 0:nreal, CC:CC + NH],
                                 in_=lr_t[:, :, :], func=ACT.Exp)
            nc.vector.tensor_tensor(
                out=g_t[:, 0:nreal, 0:CC].rearrange(
                    "p g (h c) -> p g h c", h=NH, c=CH),
                in0=g_t[:, 0:nreal, 0:CC].rearrange(
                    "p g (h c) -> p g h c", h=NH, c=CH),
                in1=g_t[:, 0:nreal, CC:CC + NH, None].to_broadcast(
                    [128, nreal, NH, CH]),
                op=mybir.AluOpType.mult)

            for blk in blocks:
                r0 = int(plan.chunk_start[blk]) - goff
                nchb = int(plan.nch[blk])
                ps = psum.tile([128, NCOL], f32, tag="agg")
                for j in range(nchb):
                    nc.tensor.matmul(ps[:, :], lhsT=ident[:, :],
                                     rhs=g_t[:, r0 + j, 0:NCOL],
                                     start=(j == 0), stop=(j == nchb - 1))

                pg = spool.tile([128, NCOL], f32, tag="pg")
                nc.scalar.activation(out=pg[:, :], in_=ps[:, :], func=ACT.Copy)
                recip = spool.tile([128, NH], f32, tag="recip")
                nc.vector.reciprocal(out=recip[:, :], in_=pg[:, CC:NCOL])
                o_t = spool.tile([128, CC], f32, tag="outb")
                if layer == 1:
                    nc.vector.tensor_tensor(
                        out=o_t[:, :].rearrange("p (h c) -> p h c",
                                                h=NH, c=CH),
                        in0=pg[:, 0:CC].rearrange("p (h c) -> p h c",
                                                  h=NH, c=CH),
                        in1=recip[:, :, None].to_broadcast([128, NH, CH]),
                        op=mybir.AluOpType.mult)

                if layer == 1:
                    obt = spool.tile([128, CC], f32, tag="outbt")
                    nc.vector.tensor_tensor(out=obt[:, :], in0=o_t[:, :],
                                            in1=bias[:, :],
                                            op=mybir.AluOpType.add)
                    ob = spool.tile([128, CC], bfl, tag="outbf")
                    nc.scalar.activation(out=ob[:, :], in_=obt[:, :],
                                         func=ACT.Relu)
                    tps = psum2.tile([D1, 128], bfl, tag="tp")
                    nc.tensor.transpose(tps[:, :], ob[:, :], ident[:, :])
                    h1T = spool.tile([D1, 128], bfl, tag="h1T")
                    nc.vector.tensor_copy(out=h1T[:, :], in_=tps[:, :])
                    h2ps = psum2.tile([128, C2], f32, tag="h2")
                    nc.tensor.matmul(h2ps[:, :], lhsT=h1T[:, :], rhs=w2_t[:, :],
                                     start=True, stop=True)
                    t2row = spool.tile([128, 48], bfl, tag="t2row")
                    nc.scalar.activation(out=t2row[:, 0:C2], in_=h2ps[:, :],
                                         func=ACT.Copy)
                    t1 = spool.tile([128, C2], f32, tag="t1")
                    nc.vector.tensor_tensor(out=t1[:, :], in0=h2ps[:, :],
                                            in1=a2s_t[:, :],
                                            op=mybir.AluOpType.mult)
                    nc.vector.tensor_reduce(
                        out=t2row[:, C2:C2 + 2].bitcast(f32), in_=t1[:, :],
                        axis=mybir.AxisListType.X, op=mybir.AluOpType.add)
                    t2 = spool.tile([128, C2], f32, tag="t2")
                    nc.vector.tensor_tensor(out=t2[:, :], in0=h2ps[:, :],
                                            in1=a2d_t[:, :],
                                            op=mybir.AluOpType.mult)
                    nc.vector.tensor_reduce(
                        out=adst_next[:, blk, :], in_=t2[:, :],
                        axis=mybir.AxisListType.X, op=mybir.AluOpType.add)
                    nc.sync.dma_start(
                        out=t_next_shard[blk * BLK:(blk + 1) * BLK, 0:48],
                        in_=t2row[:, :])
                else:
                    lg = spool.tile([128, C2], f32, tag="logits")
                    nc.vector.scalar_tensor_tensor(
                        out=lg[:, :], in0=pg[:, 0:CC], scalar=recip[:, 0:1],
                        in1=b2_t[:, :], op0=mybir.AluOpType.mult,
                        op1=mybir.AluOpType.add)
                    negm = spool.tile([128, 1], f32, tag="negm")
                    nc.vector.tensor_reduce(out=negm[:, :], in_=lg[:, :],
                                            axis=mybir.AxisListType.X,
                                            op=mybir.AluOpType.max, negate=True)
                    ex = spool.tile([128, C2], f32, tag="sfex")
                    ssum = spool.tile([128, 1], f32, tag="ssum")
                    nc.scalar.activation(out=ex[:, :], in_=lg[:, :],
                                         func=ACT.Exp,
                                         bias=negm[:, :], accum_out=ssum[:, :])
                    lse = spool.tile([128, 1], f32, tag="lse")
                    nc.scalar.activation(out=lse[:, :], in_=ssum[:, :],
                                         func=ACT.Ln)
                    res = spool.tile([128, C2], f32, tag="res")
                    nc.vector.scalar_tensor_tensor(
                        out=res[:, :], in0=lg[:, :], scalar=negm[:, :],
                        in1=lse[:, :].to_broadcast([128, C2]),
                        op0=mybir.AluOpType.add, op1=mybir.AluOpType.subtract)
                    nc.sync.dma_start(out=out_ext[blk * BLK:(blk + 1) * BLK, :],
                                      in_=res[:, :])

            last_blk = blocks[-1]
            while agq and last_blk >= ag_after[len(ag_next) - len(agq)] + AG_LAG:
                ins_ap, outs_ap = agq.pop(0)
                nc.gpsimd.collective_compute(
                    "AllGather", mybir.AluOpType.bypass, replica_groups=rg,
                    ins=[ins_ap.opt()], outs=[outs_ap.opt()])
        while agq:
            ins_ap, outs_ap = agq.pop(0)
            nc.gpsimd.collective_compute(
                "AllGather", mybir.AluOpType.bypass, replica_groups=rg,
                ins=[ins_ap.opt()], outs=[outs_ap.opt()])


def _host_inputs(x, W1, att_src1, att_dst1, b1, W2, att_src2, att_dst2, b2,
                 plan):
    w1r = np.ascontiguousarray(
        np.asarray(W1, np.float32).reshape(4, 128, D1).transpose(1, 0, 2)
    ).reshape(128, 4 * D1).astype(bf16)
    rep = lambda v, n: np.tile(np.asarray(v, np.float32).reshape(1, n),
                               (128, 1)).astype(np.float32)
    x32 = np.asarray(x, np.float32)

    # xT per core: column s = x[node at (c, s)].T
    order = plan.order
    in_maps = []
    for c in range(N_CORES):
        ranks = np.arange(SLOTS) * N_CORES + c
        valid = ranks < N_NODES
        nodes = order[ranks[valid]]
        xT = np.zeros((F_IN, SLOTS), bf16)
        xT[:, valid] = x32[nodes].T.astype(bf16)
        in_maps.append({
            "xT": xT,
            "w1r": w1r,
            "w2": np.asarray(W2, np.float32).astype(bf16),
            "a1srep": rep(att_src1, D1),
            "a1drep": rep(att_dst1, D1),
            "a2srep": rep(att_src2, C2),
            "a2drep": rep(att_dst2, C2),
            "b1rep": rep(b1, D1),
            "b2rep": rep(b2, C2),
            "idxs": plan.idx_slabs[c],
        })
    return in_maps


def kernel_run(inputs, trace=False):
    """Build (cached), run, and return (out [50000,40] f32, exec_time_ns)."""
    edge_index = inputs["edge_index"]
    plan = _prep(edge_index)

    key = tuple(plan.nch)
    if key not in _CACHE:
        _CACHE[key] = _build(plan)
    nc = _CACHE[key]

    in_maps = _host_inputs(
        inputs["x"], inputs["W1"], inputs["att_src1"], inputs["att_dst1"],
        inputs["b1"], inputs["W2"], inputs["att_src2"], inputs["att_dst2"],
        inputs["b2"], plan)

    if trace:
        _install_ntff_hook()
    res = run_bass_kernel_spmd(nc, in_maps, core_ids=list(range(N_CORES)),
                               trace=trace)
    # undo the slot permutation
    out = np.zeros((N_NODES, C2), np.float32)
    order = plan.order
    for c in range(N_CORES):
        o = res.results[c]["out"]
        ranks = np.arange(SLOTS) * N_CORES + c
        valid = ranks < N_NODES
        out[order[ranks[valid]]] = o[valid]
    return out, res.exec_time_ns


def kernel(**inputs):
    out, _ = kernel_run(inputs)
    return out
